# revision 25
# baseline (speedup 1.0000x reference)
"""Trainium2 Bass kernel for nn_DecoderStack (2-layer decoder + FFN).

B=4 T=1024 D=1024 H=16 DK=DV=64 FF=4096, fp32 I/O.

Sharding (8 cores): core c -> batch b=c//2, head-group m=c%2 (8 of 16 heads).
Activations kept transposed on device: [d on partitions, t on free].
Matmul operands are bf16 (fp32 PSUM accumulation); stats/normalization in fp32.
Cross-core: per-pair AllReduce of the Wo1 partial; per-pair ReduceScatter of
(Wo2 partial + out1/2) giving each core its residual-included t-half for the
locally computed FFN.

kernel(**inputs) takes full unsharded inputs, returns (x, out3) like the ref.
"""
import contextlib

import numpy as np

import concourse.bass as bass  # noqa: F401
import concourse.tile as tile
from concourse import bacc, mybir

F32 = mybir.dt.float32
F32R = mybir.dt.float32r
BF16 = mybir.dt.bfloat16
AF = mybir.ActivationFunctionType
ALU = mybir.AluOpType

NCORES = 8
B, T, D, H, DK, DV, FF = 4, 1024, 1024, 16, 64, 64, 4096
NCH = D // 128          # 8 d-chunks of 128
NPAIR = 4               # head-pairs per core (8 heads)
THALF = T // 2
NFT = FF // 128         # 32 f-tiles
ISQ = float(1.0 / np.sqrt(np.float32(DK)))
INV_D = float(1.0 / D)
INV_D1 = float(1.0 / (D - 1))
D_OVER_D1 = float(D / (D - 1))

REPLICA_GROUPS = [[0, 1], [2, 3], [4, 5], [6, 7]]


# ================================================================ builder ===
def build_nc(debug=False):
    nc = bacc.Bacc("TRN2", target_bir_lowering=False, debug=False,
                   num_devices=NCORES)

    io = {}

    def din(name, shape, dt):
        io[name] = nc.dram_tensor(name, shape, dt, kind="ExternalInput")

    din("yT", [NCH, 128, T], BF16)
    din("yT32", [NCH, 128, T], F32)
    din("xT", [NCH, 128, T], BF16)
    din("wq1", [NCH, 128, 512], BF16)
    din("wk1", [NCH, 128, 512], BF16)
    din("wv1", [NCH, 128, 512], BF16)
    din("wo1", [NPAIR, 128, D], BF16)
    din("wq2", [NCH, 128, 512], BF16)
    din("wk2", [NCH, 128, 512], BF16)
    din("wv2", [NCH, 128, 512], BF16)
    din("wo2", [NPAIR, 128, D], BF16)
    din("win", [NFT, NCH, 128, 128], BF16)
    din("wout", [NCH, NFT, 128, 128], BF16)
    din("bin", [128, NFT], F32)
    din("bout", [128, NCH], F32)
    din("mask", [128, 128], BF16)

    out_d = nc.dram_tensor("out3T", [NCH, 128, THALF], F32, kind="ExternalOutput")
    dbg = {}
    if debug:
        for name, shape, dt in (
                ("d_out1T", [128, NCH, T], BF16),
                ("d_qt1", [128, NPAIR, T], BF16),
                ("d_kt1", [128, NPAIR, T], BF16),
                ("d_vv1", [128, NCH, 512], BF16),
                ("d_aot1", [128, NPAIR, T], BF16),
                ("d_out2T", [128, NCH, THALF], BF16),
                ("d_z2", [128, NCH, THALF], BF16)):
            dbg[name] = nc.dram_tensor(name, shape, dt, kind="ExternalOutput")

    with tile.TileContext(nc) as tc:
        _emit(nc, tc, io, out_d, dbg)
    nc.compile()
    return nc


def _dump(nc, dbg, name, t_sb):
    if name in dbg:
        # stage through fp32 copy in DRAM-compatible layout
        nc.sync.dma_start(dbg[name].ap(), t_sb)


def _emit(nc, tc, io, out_d, dbg):
    ctx = contextlib.ExitStack()
    with ctx:
        # ---------------- outer pools (live whole kernel) ----------------
        const = ctx.enter_context(tc.tile_pool(name="const", bufs=1))
        stat = ctx.enter_context(tc.tile_pool(name="stat", bufs=1))
        scr = ctx.enter_context(tc.tile_pool(name="scr", bufs=1))
        epool = ctx.enter_context(tc.tile_pool(name="epool", bufs=1))
        hz = ctx.enter_context(tc.tile_pool(name="hz", bufs=1))
        dram = ctx.enter_context(tc.tile_pool(name="dram", bufs=1, space="DRAM"))
        psp = ctx.enter_context(tc.tile_pool(name="psp", bufs=1, space="PSUM"))

        def ps_tile(shape=(128, T)):
            return psp.tile(list(shape), F32, tag="ps", bufs=4, name="ps")

        # ---------------- constants ----------------
        ones_col = const.tile([128, 1], BF16)
        nc.vector.memset(ones_col[:], 1.0)
        ones_row = const.tile([1, 128], F32)
        nc.vector.memset(ones_row[:], 1.0)
        one1 = const.tile([1, 1], F32)
        nc.vector.memset(one1[:], 1.0)
        mask_sb = const.tile([128, 128], BF16)
        nc.sync.dma_start(mask_sb[:], io["mask"].ap())
        bin_sb = const.tile([128, NFT], F32)
        nc.sync.dma_start(bin_sb[:], io["bin"].ap())
        bout_sb = const.tile([128, NCH], F32)
        nc.sync.dma_start(bout_sb[:], io["bout"].ap())

        # =========== transposed-space layer norm (over d = partitions) =====
        def norm_into(z_sb, tw, out_sb, chunk_writer=None, off=0,
                      apply_src=None):
            """out_sb[:, c, :tw] = (z - mean)/std per t-column. z_sb bf16
            [128, NCH, tw]; out_sb may alias z_sb (chunk-wise in-place).
            If chunk_writer is given, per-chunk f32 results are handed to it
            instead of writing out_sb."""
            s_ps = ps_tile((1, T))
            for c in range(NCH):
                for nh in range(0, tw, 512):
                    w = min(512, tw - nh)
                    nc.tensor.matmul(
                        s_ps[:, nh:nh + w], ones_col[:],
                        z_sb[:, c, off + nh:off + nh + w],
                        start=(c == 0), stop=(c == NCH - 1))
            ss_ps = ps_tile((1, T))
            for c in range(NCH):
                zsq = scr.tile([128, T], BF16, tag="zsq", bufs=2, name="zsq")
                nc.vector.tensor_mul(out=zsq[:, :tw],
                                     in0=z_sb[:, c, off:off + tw],
                                     in1=z_sb[:, c, off:off + tw])
                for nh in range(0, tw, 512):
                    w = min(512, tw - nh)
                    nc.tensor.matmul(
                        ss_ps[:, nh:nh + w], ones_col[:], zsq[:, nh:nh + w],
                        start=(c == 0), stop=(c == NCH - 1))
            mean = stat.tile([1, T], F32, tag="mean", bufs=1, name="mean")
            nc.vector.tensor_scalar(out=mean[:, :tw], in0=s_ps[:, :tw],
                                    scalar1=INV_D, scalar2=None, op0=ALU.mult)
            m2 = stat.tile([1, T], F32, tag="m2", bufs=1, name="m2")
            nc.vector.tensor_mul(out=m2[:, :tw], in0=mean[:, :tw],
                                 in1=mean[:, :tw])
            var = stat.tile([1, T], F32, tag="var", bufs=1, name="var")
            nc.vector.tensor_scalar(out=var[:, :tw], in0=ss_ps[:, :tw],
                                    scalar1=INV_D1, scalar2=None, op0=ALU.mult)
            nc.vector.tensor_scalar(out=m2[:, :tw], in0=m2[:, :tw],
                                    scalar1=D_OVER_D1, scalar2=None,
                                    op0=ALU.mult)
            nc.vector.tensor_tensor(var[:, :tw], var[:, :tw], m2[:, :tw],
                                    ALU.subtract)
            nc.scalar.activation(out=var[:, :tw], in_=var[:, :tw], func=AF.Sqrt)
            rstd = stat.tile([1, T], F32, tag="rstd", bufs=1, name="rstd")
            nc.vector.reciprocal(out=rstd[:, :tw], in_=var[:, :tw])
            nmr = stat.tile([1, T], F32, tag="nmr", bufs=1, name="nmr")
            nc.vector.tensor_mul(out=nmr[:, :tw], in0=mean[:, :tw],
                                 in1=rstd[:, :tw])
            nc.vector.tensor_scalar(out=nmr[:, :tw], in0=nmr[:, :tw],
                                    scalar1=-1.0, scalar2=None, op0=ALU.mult)
            rstd_ps = ps_tile()
            nmr_ps = ps_tile()
            for nh in range(0, tw, 512):
                w = min(512, tw - nh)
                nc.tensor.matmul(rstd_ps[:, nh:nh + w], ones_row[:],
                                 rstd[:, nh:nh + w], start=True, stop=True)
                nc.tensor.matmul(nmr_ps[:, nh:nh + w], ones_row[:],
                                 nmr[:, nh:nh + w], start=True, stop=True)
            rstd_bc = scr.tile([128, T], F32, tag="bc", bufs=2, name="rstd_bc")
            nc.vector.tensor_copy(out=rstd_bc[:, :tw], in_=rstd_ps[:, :tw])
            nmr_bc = scr.tile([128, T], F32, tag="bc", bufs=2, name="nmr_bc")
            nc.vector.tensor_copy(out=nmr_bc[:, :tw], in_=nmr_ps[:, :tw])
            a_src = z_sb if apply_src is None else apply_src
            for c in range(NCH):
                tmp = scr.tile([128, T], F32, tag="s4", bufs=4, name="ntmp")
                nc.vector.tensor_mul(out=tmp[:, :tw],
                                     in0=a_src[:, c, off:off + tw],
                                     in1=rstd_bc[:, :tw])
                if chunk_writer is None:
                    nc.vector.tensor_tensor(out_sb[:, c, off:off + tw], tmp[:, :tw],
                                            nmr_bc[:, :tw], ALU.add)
                else:
                    oc = scr.tile([128, T], F32, tag="s4", bufs=4, name="oc")
                    nc.vector.tensor_tensor(oc[:, :tw], tmp[:, :tw],
                                            nmr_bc[:, :tw], ALU.add)
                    chunk_writer(c, oc[:, :tw])

        # ================= attention inner block (scores/exp/AV) ===========
        # proj_spec: list of (dst, w_t, src, fold) projected per-pair right
        # before that pair's scores — keeps PE dense while ACT runs exps.
        def attn_inner(qt_sb, kt_sb, vv_sb, aot_sb, proj_spec=()):
            for p in range(NPAIR):
                for dst, w_t, src, fld in proj_spec:
                    pp = ps_tile()
                    for c in range(NCH):
                        lhsT = w_t[:, c, 128 * p:128 * (p + 1)]
                        for nh in range(2):
                            nc.tensor.matmul(
                                pp[:, 512 * nh:512 * (nh + 1)], lhsT,
                                src[:, c, 512 * nh:512 * (nh + 1)],
                                start=(c == 0), stop=(c == NCH - 1))
                    if fld is None:
                        nc.vector.tensor_copy(out=dst[:, p, :], in_=pp[:])
                    else:
                        nc.vector.tensor_scalar(
                            out=dst[:, p, :], in0=pp[:], scalar1=fld[p][:],
                            scalar2=None, op0=ALU.mult)
                av_ps = ps_tile()
                for st in range(NCH):
                    zp = stat.tile([128, 2], F32, tag="zp", bufs=4, name="zp")
                    e_pair = []
                    for h in range(2):
                        sc_ps = ps_tile()
                        k0 = 64 * h
                        lhsT = kt_sb[k0:k0 + 64, p, 128 * st:128 * (st + 1)]
                        for nh in range(2):
                            nc.tensor.matmul(
                                sc_ps[:, 512 * nh:512 * (nh + 1)], lhsT,
                                qt_sb[k0:k0 + 64, p, 512 * nh:512 * (nh + 1)],
                                start=True, stop=True, tile_position=(k0, 0))
                        e_st = epool.tile([128, T], BF16, tag="E", bufs=3,
                                          name="e_st")
                        nc.scalar.activation(
                            out=e_st[:], in_=sc_ps[:], func=AF.Exp, scale=ISQ,
                            accum_out=zp[:, h:h + 1])
                        e_pair.append(e_st)
                    rp = stat.tile([128, 2], F32, tag="rp", bufs=4, name="rp")
                    nc.vector.reciprocal(out=rp[:], in_=zp[:])
                    vv_sc = scr.tile([128, 2, 64], BF16, tag="vvsc", bufs=3,
                                     name="vv_sc")
                    nc.vector.tensor_tensor(
                        vv_sc[:],
                        vv_sb[:, st, 128 * p:128 * (p + 1)].rearrange(
                            "s (h v) -> s h v", h=2),
                        rp[:, :, None].to_broadcast([128, 2, 64]),
                        ALU.mult)
                    for h in range(2):
                        for nh in range(2):
                            nc.tensor.matmul(
                                av_ps[64 * h:64 * (h + 1),
                                      512 * nh:512 * (nh + 1)],
                                vv_sc[:, h, :],
                                e_pair[h][:, 512 * nh:512 * (nh + 1)],
                                start=(st == 0), stop=(st == NCH - 1),
                                tile_position=(0, 64 * h))
                nc.vector.tensor_copy(out=aot_sb[:, p, :], in_=av_ps[:])

        # ================= projection helpers ==============================
        def proj_qk(dst, w_t, src, fold):
            """dst[:, p, :] = (W_pair^T @ src) [optionally * fold[p] rows]."""
            for p in range(NPAIR):
                pp = ps_tile()
                for c in range(NCH):
                    lhsT = w_t[:, c, 128 * p:128 * (p + 1)]
                    for nh in range(2):
                        nc.tensor.matmul(
                            pp[:, 512 * nh:512 * (nh + 1)], lhsT,
                            src[:, c, 512 * nh:512 * (nh + 1)],
                            start=(c == 0), stop=(c == NCH - 1))
                if fold is None:
                    nc.vector.tensor_copy(out=dst[:, p, :], in_=pp[:])
                else:
                    nc.vector.tensor_scalar(
                        out=dst[:, p, :], in0=pp[:], scalar1=fold[p][:],
                        scalar2=None, op0=ALU.mult)

        def proj_v(dst, w_t, src):
            for st in range(NCH):
                vp = ps_tile((128, 512))
                for c in range(NCH):
                    nc.tensor.matmul(
                        vp[:], src[:, c, 128 * st:128 * (st + 1)],
                        w_t[:, c, :], start=(c == 0), stop=(c == NCH - 1))
                nc.vector.tensor_copy(out=dst[:, st, :], in_=vp[:])

        def wo_partial(wo_sb, aot_sb, emit_e_tile):
            for e in range(NCH):
                wo_ps = ps_tile()
                for p in range(NPAIR):
                    lhsT = wo_sb[:, p, 128 * e:128 * (e + 1)]
                    for nh in range(2):
                        nc.tensor.matmul(
                            wo_ps[:, 512 * nh:512 * (nh + 1)], lhsT,
                            aot_sb[:, p, 512 * nh:512 * (nh + 1)],
                            start=(p == 0), stop=(p == NPAIR - 1))
                emit_e_tile(e, wo_ps)

        def load_w(pool, name, tag):
            t = pool.tile([128, NCH, 512], BF16, tag=tag, name=name + "_sb")
            nc.sync.dma_start(t[:], io[name].ap().rearrange("c p k -> p c k"))
            return t

        # ============================ start =================================
        with tc.tile_pool(name="actA", bufs=1) as actA:
            y_sb = actA.tile([128, NCH, T], BF16, tag="y", name="y_sb")

            with tc.tile_pool(name="gio", bufs=1) as gio:
                qt = gio.tile([128, NPAIR, T], BF16, tag="qt", name="qt")
                kt = gio.tile([128, NPAIR, T], BF16, tag="kt", name="kt")
                vv = gio.tile([128, NCH, 512], BF16, tag="vv", name="vv")
                aot = gio.tile([128, NPAIR, T], BF16, tag="aot", name="aot")

                ar_in = dram.tile([NCH, 128, T], BF16, tag="ar_in", name="ar_in")
                ar_out = dram.tile([NCH, 128, T], BF16, tag="ar_out",
                                   name="ar_out")
                TQ = THALF // 2
                rs_inA = dram.tile([2, NCH, 128, TQ], BF16, tag="rs_inA",
                                   name="rs_inA")
                rs_inB = dram.tile([2, NCH, 128, TQ], BF16, tag="rs_inB",
                                   name="rs_inB")
                rs_outA = dram.tile([NCH, 128, TQ], BF16, tag="rs_outA",
                                    name="rs_outA")
                rs_outB = dram.tile([NCH, 128, TQ], BF16, tag="rs_outB",
                                    name="rs_outB")

                # ---------------- Layer 1 ----------------
                with tc.tile_pool(name="w1", bufs=1) as w1:
                    # masked weight softmax for Wq1/Wk1 (no max-subtraction)
                    ewq = w1.tile([128, NCH, 512], BF16, tag="ewq", name="ewq")
                    ewk = w1.tile([128, NCH, 512], BF16, tag="ewk", name="ewk")
                    for nm, ew in (("wq1", ewq), ("wk1", ewk)):
                        raw = w1.tile([128, NCH, 512], BF16, tag="wraw",
                                      bufs=1, name="wraw")
                        nc.sync.dma_start(
                            raw[:], io[nm].ap().rearrange("c p k -> p c k"))
                        nc.scalar.activation(out=ew[:], in_=raw[:], func=AF.Exp)
                        nc.vector.tensor_tensor(
                            ew[:, 0, :].rearrange("p (q k) -> p q k", q=NPAIR),
                            ew[:, 0, :].rearrange("p (q k) -> p q k", q=NPAIR),
                            mask_sb[:, None, :].to_broadcast([128, NPAIR, 128]),
                            ALU.mult)
                    nc.sync.dma_start(
                        y_sb[:], io["yT"].ap().rearrange("c p t -> p c t"))
                    # column sums over d -> ck = 1/(Sq*Sk), transposed per pair
                    sq_ps = ps_tile((1, 512))
                    for c in range(NCH):
                        nc.tensor.matmul(sq_ps[:], ones_col[:], ewq[:, c, :],
                                         start=(c == 0), stop=(c == NCH - 1))
                    sk_ps = ps_tile((1, 512))
                    for c in range(NCH):
                        nc.tensor.matmul(sk_ps[:], ones_col[:], ewk[:, c, :],
                                         start=(c == 0), stop=(c == NCH - 1))
                    rq = stat.tile([1, 512], F32, tag="rq", name="rq")
                    rk = stat.tile([1, 512], F32, tag="rk", name="rk")
                    nc.vector.reciprocal(out=rq[:], in_=sq_ps[:])
                    nc.vector.reciprocal(out=rk[:], in_=sk_ps[:])
                    ckk = stat.tile([1, 512], F32, tag="ck", name="ckk")
                    nc.vector.tensor_mul(out=ckk[:], in0=rq[:], in1=rk[:])
                    ckT = []
                    for p in range(NPAIR):
                        ct_ps = ps_tile((128, 1))
                        nc.tensor.matmul(ct_ps[:],
                                         ckk[:, 128 * p:128 * (p + 1)],
                                         one1[:], start=True, stop=True)
                        ct = stat.tile([128, 1], F32, tag=f"ckT{p}",
                                       name=f"ckT{p}")
                        nc.vector.tensor_copy(out=ct[:], in_=ct_ps[:])
                        ckT.append(ct)

                    wv1 = load_w(w1, "wv1", "wv")
                    proj_v(vv, wv1, y_sb)
                    wo1 = w1.tile([128, NPAIR, D], BF16, tag="wo", name="wo1")
                    nc.sync.dma_start(
                        wo1[:], io["wo1"].ap().rearrange("q p e -> p q e"))

                    proj_qk(qt, ewq, y_sb, None)
                    proj_qk(kt, ewk, y_sb, ckT)
                    attn_inner(qt, kt, vv, aot)
                    _dump(nc, dbg, "d_qt1", qt[:])
                    _dump(nc, dbg, "d_kt1", kt[:])
                    _dump(nc, dbg, "d_vv1", vv[:])
                    _dump(nc, dbg, "d_aot1", aot[:])

                    def emit_ar(e, wo_ps):
                        stg = scr.tile([128, T], BF16, tag="sb4", bufs=4,
                                       name="ar_stg")
                        nc.vector.tensor_copy(out=stg[:], in_=wo_ps[:])
                        nc.sync.dma_start(ar_in[e], stg[:])

                    wo_partial(wo1, aot, emit_ar)
                nc.gpsimd.collective_compute(
                    "AllReduce", ALU.add, replica_groups=REPLICA_GROUPS,
                    ins=[ar_in.opt()], outs=[ar_out.opt()])

                # ---------------- Layer 2 (overlaps the AllReduce) ---------
                with tc.tile_pool(name="actB", bufs=1) as actB:
                    x_sb = actB.tile([128, NCH, T], BF16, tag="x", name="x_sb")
                    nc.sync.dma_start(
                        x_sb[:], io["xT"].ap().rearrange("c p t -> p c t"))
                    with tc.tile_pool(name="w2", bufs=1) as w2:
                        wq2 = load_w(w2, "wq2", "wq2")
                        wk2 = load_w(w2, "wk2", "wk2")
                        wv2 = load_w(w2, "wv2", "wv2")
                        wo2 = w2.tile([128, NPAIR, D], BF16, tag="wo2",
                                      name="wo2")
                        nc.sync.dma_start(
                            wo2[:], io["wo2"].ap().rearrange("q p e -> p q e"))
                        # K/V projections only need x -> run during the AR
                        proj_qk(kt, wk2, x_sb, None)
                        proj_v(vv, wv2, x_sb)

                        # z1 = AR(out1_pre) + y(fp32) ; out1 = norm(z1) in-place
                        for c in range(NCH):
                            rb = scr.tile([128, T], BF16, tag="sb4", bufs=4,
                                          name="rb")
                            nc.sync.dma_start(rb[:], ar_out[c])
                            yf = scr.tile([128, T], F32, tag="s4", bufs=4,
                                          name="yf")
                            nc.sync.dma_start(yf[:], io["yT32"].ap()[c])
                            nc.vector.tensor_tensor(y_sb[:, c, :], rb[:],
                                                    yf[:], ALU.add)
                        out1 = y_sb  # alias: z1 normalized in place
                        norm_into(y_sb, T, out1)
                        _dump(nc, dbg, "d_out1T", out1[:])

                        proj_qk(qt, wq2, out1, None)
                        attn_inner(qt, kt, vv, aot)

                        def emit_rs(e, wo_ps):
                            half = scr.tile([128, T], BF16, tag="sb4", bufs=4,
                                            name="half")
                            nc.vector.tensor_scalar(
                                out=half[:], in0=out1[:, e, :], scalar1=0.5,
                                scalar2=None, op0=ALU.mult)
                            res = scr.tile([128, T], BF16, tag="sb4", bufs=4,
                                           name="res")
                            nc.vector.tensor_tensor(res[:], wo_ps[:], half[:],
                                                    ALU.add)
                            nc.sync.dma_start(rs_inA[0, e], res[:, 0:TQ])
                            nc.sync.dma_start(rs_inA[1, e],
                                              res[:, THALF:THALF + TQ])
                            nc.sync.dma_start(rs_inB[0, e], res[:, TQ:THALF])
                            nc.sync.dma_start(rs_inB[1, e], res[:, THALF + TQ:])

                        wo_partial(wo2, aot, emit_rs)
                nc.gpsimd.collective_compute(
                    "ReduceScatter", ALU.add, replica_groups=REPLICA_GROUPS,
                    ins=[rs_inA.opt()], outs=[rs_outA.opt()])
                nc.gpsimd.collective_compute(
                    "ReduceScatter", ALU.add, replica_groups=REPLICA_GROUPS,
                    ins=[rs_inB.opt()], outs=[rs_outB.opt()])

            # ---------------- FFN on local t-half ----------------
            with tc.tile_pool(name="fh", bufs=1) as fh:
                h_sb = fh.tile([128, NFT, THALF], BF16, tag="h", name="h_sb")
                NWIN = 4
                pre_win = {}
                for ft in range(NWIN):
                    wt = fh.tile([128, NCH, 128], BF16, tag="win",
                                 bufs=NWIN, name="win_t")
                    nc.sync.dma_start(
                        wt[:], io["win"].ap()[ft].rearrange("c p f -> p c f"))
                    pre_win[ft] = wt
                pre_wout = fh.tile([128, NFT, 128], BF16, tag="wout", bufs=2,
                                   name="wout_t")
                nc.sync.dma_start(pre_wout[:],
                                  io["wout"].ap()[0].rearrange("f p m -> p f m"))

                TQ = THALF // 2
                z2 = hz.tile([128, NCH, THALF], BF16, tag="hz", bufs=1, name="z2")
                nc.sync.dma_start(z2[:, :, 0:TQ],
                                  rs_outA.rearrange("c p t -> p c t"))
                out2 = z2
                norm_into(z2, TQ, out2, off=0)
                nc.sync.dma_start(z2[:, :, TQ:THALF],
                                  rs_outB.rearrange("c p t -> p c t"))
                win_tiles = {}
                for ft in range(NFT):
                    if ft in pre_win:
                        win_t = pre_win.pop(ft)
                    else:
                        win_t = fh.tile([128, NCH, 128], BF16, tag="win",
                                        bufs=NWIN, name="win_t")
                        nc.sync.dma_start(
                            win_t[:],
                            io["win"].ap()[ft].rearrange("c p f -> p c f"))
                    win_tiles[ft] = win_t
                    hp = ps_tile((128, THALF))
                    for c in range(NCH):
                        nc.tensor.matmul(hp[:, 0:TQ], win_t[:, c, :],
                                         out2[:, c, 0:TQ],
                                         start=(c == 0), stop=(c == NCH - 1))
                    nc.scalar.activation(out=h_sb[:, ft, 0:TQ], in_=hp[:, 0:TQ],
                                         func=AF.Relu,
                                         bias=bin_sb[:, ft:ft + 1], scale=1.0)
                    if ft == 0:
                        norm_into(z2, TQ, out2, off=TQ)
                    hp2 = ps_tile((128, THALF))
                    for c in range(NCH):
                        nc.tensor.matmul(hp2[:, 0:TQ], win_t[:, c, :],
                                         out2[:, c, TQ:THALF],
                                         start=(c == 0), stop=(c == NCH - 1))
                    nc.scalar.activation(out=h_sb[:, ft, TQ:THALF],
                                         in_=hp2[:, 0:TQ], func=AF.Relu,
                                         bias=bin_sb[:, ft:ft + 1], scale=1.0)
                    del win_tiles[ft]

                z3 = out2  # in-place: out2[:, e] is last read by this add
                z3f = fh.tile([128, NCH, THALF], F32, tag="z3f", name="z3f")
                for e in range(NCH):
                    if e == 0:
                        wout_t = pre_wout
                    else:
                        wout_t = fh.tile([128, NFT, 128], BF16,
                                         tag="wout", bufs=2, name="wout_t")
                        nc.sync.dma_start(
                            wout_t[:],
                            io["wout"].ap()[e].rearrange("f p m -> p f m"))
                    fp = ps_tile((128, THALF))
                    for fc in range(NFT):
                        nc.tensor.matmul(
                            fp[:], wout_t[:, fc, :], h_sb[:, fc, :],
                            start=(fc == 0), stop=(fc == NFT - 1))
                    t1 = scr.tile([128, T], F32, tag="s4", bufs=4, name="fftmp")
                    nc.vector.tensor_scalar(out=t1[:, :THALF], in0=fp[:],
                                            scalar1=bout_sb[:, e:e + 1],
                                            scalar2=None, op0=ALU.add)
                    nc.vector.tensor_tensor(z3f[:, e, :], t1[:, :THALF],
                                            out2[:, e, :], ALU.add)
                    nc.vector.tensor_copy(out=z3[:, e, :], in_=z3f[:, e, :])
                norm_into(z3, THALF, None,
                          chunk_writer=lambda c, oc: nc.sync.dma_start(
                              out_d.ap()[c], oc), apply_src=z3f)


# ============================================================== host side ===
def _to_bf16(a):
    import ml_dtypes
    return np.asarray(a, np.float32).astype(ml_dtypes.bfloat16)


def _prep_inputs(inputs):
    """Per-core in_maps (host does transposes/tiling/dtype casts only)."""
    x = np.asarray(inputs["x"], np.float32)
    y = np.asarray(inputs["y"], np.float32)
    mask = _to_bf16(np.tile(np.tril(np.ones((128, DK), np.float32)), (1, 2)))
    win_t = _to_bf16(np.asarray(inputs["w_in"], np.float32).T
                     .reshape(NCH, 128, NFT, 128).transpose(2, 0, 1, 3))
    wout_t = _to_bf16(np.asarray(inputs["w_out"], np.float32).T
                      .reshape(NFT, 128, NCH, 128).transpose(2, 0, 1, 3))
    bin2 = np.ascontiguousarray(
        np.asarray(inputs["b_in"], np.float32).reshape(NFT, 128).T)
    bout2 = np.ascontiguousarray(
        np.asarray(inputs["b_out"], np.float32).reshape(NCH, 128).T)

    def packw(w, hs):
        return _to_bf16(np.asarray(w, np.float32)[hs].transpose(1, 0, 2)
                        .reshape(D, 512).reshape(NCH, 128, 512))

    shared = {"win": win_t, "wout": wout_t, "bin": bin2, "bout": bout2,
              "mask": mask}
    in_maps = []
    for c in range(NCORES):
        b, m = c // 2, c % 2
        hs = slice(8 * m, 8 * (m + 1))
        im = dict(shared)
        im["yT"] = _to_bf16(y[b].T.reshape(NCH, 128, T))
        im["yT32"] = np.ascontiguousarray(y[b].T.reshape(NCH, 128, T))
        im["xT"] = _to_bf16(x[b].T.reshape(NCH, 128, T))
        im["wq1"] = packw(inputs["Wq1"], hs)
        im["wk1"] = packw(inputs["Wk1"], hs)
        im["wv1"] = packw(inputs["Wv1"], hs)
        im["wq2"] = packw(inputs["Wq2"], hs)
        im["wk2"] = packw(inputs["Wk2"], hs)
        im["wv2"] = packw(inputs["Wv2"], hs)
        im["wo1"] = _to_bf16(np.asarray(inputs["Wo1"], np.float32)
                             [512 * m:512 * (m + 1)].reshape(NPAIR, 128, D))
        im["wo2"] = _to_bf16(np.asarray(inputs["Wo2"], np.float32)
                             [512 * m:512 * (m + 1)].reshape(NPAIR, 128, D))
        in_maps.append(im)
    return in_maps


def _assemble(results):
    out3 = np.empty((B, T, D), np.float32)
    for b in range(B):
        halves = [results[2 * b + m]["out3T"].reshape(D, THALF)
                  for m in range(2)]
        out3[b] = np.concatenate(halves, axis=1).T
    return out3


# ================================================================ runner ===
_CACHE = {}


def _make_runner(nc, n_cores):
    import jax
    from jax.sharding import Mesh, PartitionSpec
    from jax.experimental.shard_map import shard_map
    from concourse.bass2jax import (_bass_exec_p, install_neuronx_cc_hook,
                                    partition_id_tensor)

    install_neuronx_cc_hook()
    partition_name = nc.partition_id_tensor.name if nc.partition_id_tensor else None
    in_names, out_names, out_avals, zero_outs = [], [], [], []
    for alloc in nc.m.functions[0].allocations:
        if not isinstance(alloc, mybir.MemoryLocationSet):
            continue
        name = alloc.memorylocations[0].name
        if alloc.kind == "ExternalInput":
            if name != partition_name:
                in_names.append(name)
        elif alloc.kind == "ExternalOutput":
            shape = tuple(alloc.tensor_shape)
            dtype = mybir.dt.np(alloc.dtype)
            out_names.append(name)
            out_avals.append(jax.core.ShapedArray(shape, dtype))
            zero_outs.append(np.zeros(shape, dtype))
    n_params = len(in_names)
    n_outs = len(out_avals)
    all_in = in_names + out_names + ([partition_name] if partition_name else [])

    def _body(*args):
        operands = list(args)
        if partition_name is not None:
            operands.append(partition_id_tensor())
        return tuple(_bass_exec_p.bind(
            *operands, out_avals=tuple(out_avals), in_names=tuple(all_in),
            out_names=tuple(out_names), lowering_input_output_aliases=(),
            sim_require_finite=True, sim_require_nnan=True, nc=nc))

    devices = jax.devices()[:n_cores]
    mesh = Mesh(np.asarray(devices), ("core",))
    sharded = jax.jit(
        shard_map(_body, mesh=mesh,
                  in_specs=(PartitionSpec("core"),) * (n_params + n_outs),
                  out_specs=(PartitionSpec("core"),) * n_outs,
                  check_rep=False),
        keep_unused=True)

    def run(in_maps):
        concat_in = [
            np.concatenate([np.asarray(in_maps[c][nm]) for c in range(n_cores)],
                           axis=0)
            for nm in in_names
        ]
        concat_zero = [np.concatenate([z] * n_cores, axis=0) for z in zero_outs]
        outs = [np.asarray(o) for o in sharded(*concat_in, *concat_zero)]
        results = []
        for c in range(n_cores):
            r = {}
            for i, nm in enumerate(out_names):
                per = outs[i].shape[0] // n_cores
                r[nm] = outs[i][c * per:(c + 1) * per]
            results.append(r)
        return results

    return run


def _get_built(debug=False):
    key = "dbg" if debug else "main"
    if key not in _CACHE:
        nc = build_nc(debug=debug)
        run = _make_runner(nc, NCORES)
        _CACHE[key] = (nc, run)
    return _CACHE[key]


def kernel(**inputs):
    nc, run = _get_built()
    in_maps = _prep_inputs(inputs)
    results = run(in_maps)
    out3 = _assemble(results)
    return (np.asarray(inputs["x"], np.float32), out3)


# revision 28
# speedup vs baseline: 1.0127x; 1.0127x over previous
"""Trainium2 Bass kernel for nn_DecoderStack (2-layer decoder + FFN).

B=4 T=1024 D=1024 H=16 DK=DV=64 FF=4096, fp32 I/O.

Sharding (8 cores): core c -> batch b=c//2, head-group m=c%2 (8 of 16 heads).
Activations kept transposed on device: [d on partitions, t on free].
Matmul operands are bf16 (fp32 PSUM accumulation); stats/normalization in fp32.
Cross-core: per-pair AllReduce of the Wo1 partial; per-pair ReduceScatter of
(Wo2 partial + out1/2) giving each core its residual-included t-half for the
locally computed FFN.

kernel(**inputs) takes full unsharded inputs, returns (x, out3) like the ref.
"""
import contextlib

import numpy as np

import concourse.bass as bass  # noqa: F401
import concourse.tile as tile
from concourse import bacc, mybir

F32 = mybir.dt.float32
F32R = mybir.dt.float32r
BF16 = mybir.dt.bfloat16
AF = mybir.ActivationFunctionType
ALU = mybir.AluOpType

NCORES = 8
B, T, D, H, DK, DV, FF = 4, 1024, 1024, 16, 64, 64, 4096
NCH = D // 128          # 8 d-chunks of 128
NPAIR = 4               # head-pairs per core (8 heads)
THALF = T // 2
NFT = FF // 128         # 32 f-tiles
ISQ = float(1.0 / np.sqrt(np.float32(DK)))
INV_D = float(1.0 / D)
INV_D1 = float(1.0 / (D - 1))
D_OVER_D1 = float(D / (D - 1))

REPLICA_GROUPS = [[0, 1], [2, 3], [4, 5], [6, 7]]


# ================================================================ builder ===
def build_nc(debug=False):
    nc = bacc.Bacc("TRN2", target_bir_lowering=False, debug=False,
                   num_devices=NCORES)

    io = {}

    def din(name, shape, dt):
        io[name] = nc.dram_tensor(name, shape, dt, kind="ExternalInput")

    din("yT", [NCH, 128, T], BF16)
    din("yT32", [NCH, 128, T], F32)
    din("xT", [NCH, 128, T], BF16)
    din("wq1", [NCH, 128, 512], BF16)
    din("wk1", [NCH, 128, 512], BF16)
    din("wv1", [NCH, 128, 512], BF16)
    din("wo1", [2 * NPAIR, 128, D], BF16)
    din("wq2", [NCH, 128, 512], BF16)
    din("wk2", [NCH, 128, 512], BF16)
    din("wv2", [NCH, 128, 512], BF16)
    din("wo2", [NPAIR, 128, D], BF16)
    din("win", [NFT, NCH, 128, 128], BF16)
    din("wout", [NCH, NFT, 128, 128], BF16)
    din("bin", [128, NFT], F32)
    din("bout", [128, NCH], F32)
    din("mask", [128, 128], BF16)

    out_d = nc.dram_tensor("out3T", [NCH, 128, THALF], F32, kind="ExternalOutput")
    dbg = {}
    if debug:
        for name, shape, dt in (
                ("d_out1T", [128, NCH, T], BF16),
                ("d_qt1", [128, NPAIR, T], BF16),
                ("d_kt1", [128, NPAIR, T], BF16),
                ("d_vv1", [128, NCH, 512], BF16),
                ("d_aot1", [128, NPAIR, T], BF16),
                ("d_out2T", [128, NCH, THALF], BF16),
                ("d_z2", [128, NCH, THALF], BF16)):
            dbg[name] = nc.dram_tensor(name, shape, dt, kind="ExternalOutput")

    with tile.TileContext(nc) as tc:
        _emit(nc, tc, io, out_d, dbg)
    nc.compile()
    return nc


def _dump(nc, dbg, name, t_sb):
    if name in dbg:
        # stage through fp32 copy in DRAM-compatible layout
        nc.sync.dma_start(dbg[name].ap(), t_sb)


def _emit(nc, tc, io, out_d, dbg):
    ctx = contextlib.ExitStack()
    with ctx:
        # ---------------- outer pools (live whole kernel) ----------------
        const = ctx.enter_context(tc.tile_pool(name="const", bufs=1))
        stat = ctx.enter_context(tc.tile_pool(name="stat", bufs=1))
        scr = ctx.enter_context(tc.tile_pool(name="scr", bufs=1))
        epool = ctx.enter_context(tc.tile_pool(name="epool", bufs=1))
        hz = ctx.enter_context(tc.tile_pool(name="hz", bufs=1))
        dram = ctx.enter_context(tc.tile_pool(name="dram", bufs=1, space="DRAM"))
        psp = ctx.enter_context(tc.tile_pool(name="psp", bufs=1, space="PSUM"))

        def ps_tile(shape=(128, T)):
            return psp.tile(list(shape), F32, tag="ps", bufs=4, name="ps")

        # ---------------- constants ----------------
        ones_col = const.tile([128, 1], BF16)
        nc.vector.memset(ones_col[:], 1.0)
        ones_row = const.tile([1, 128], F32)
        nc.vector.memset(ones_row[:], 1.0)
        one1 = const.tile([1, 1], F32)
        nc.vector.memset(one1[:], 1.0)
        mask_sb = const.tile([128, 128], BF16)
        nc.sync.dma_start(mask_sb[:], io["mask"].ap())
        bin_sb = const.tile([128, NFT], F32)
        nc.sync.dma_start(bin_sb[:], io["bin"].ap())
        bout_sb = const.tile([128, NCH], F32)
        nc.sync.dma_start(bout_sb[:], io["bout"].ap())

        # =========== transposed-space layer norm (over d = partitions) =====
        def norm_into(z_sb, tw, out_sb, chunk_writer=None, off=0,
                      apply_src=None):
            """out_sb[:, c, :tw] = (z - mean)/std per t-column. z_sb bf16
            [128, NCH, tw]; out_sb may alias z_sb (chunk-wise in-place).
            If chunk_writer is given, per-chunk f32 results are handed to it
            instead of writing out_sb."""
            s_ps = ps_tile((1, T))
            for c in range(NCH):
                for nh in range(0, tw, 512):
                    w = min(512, tw - nh)
                    nc.tensor.matmul(
                        s_ps[:, nh:nh + w], ones_col[:],
                        z_sb[:, c, off + nh:off + nh + w],
                        start=(c == 0), stop=(c == NCH - 1))
            ss_ps = ps_tile((1, T))
            for c in range(NCH):
                zsq = scr.tile([128, T], BF16, tag="zsq", bufs=2, name="zsq")
                nc.vector.tensor_mul(out=zsq[:, :tw],
                                     in0=z_sb[:, c, off:off + tw],
                                     in1=z_sb[:, c, off:off + tw])
                for nh in range(0, tw, 512):
                    w = min(512, tw - nh)
                    nc.tensor.matmul(
                        ss_ps[:, nh:nh + w], ones_col[:], zsq[:, nh:nh + w],
                        start=(c == 0), stop=(c == NCH - 1))
            mean = stat.tile([1, T], F32, tag="mean", bufs=1, name="mean")
            nc.vector.tensor_scalar(out=mean[:, :tw], in0=s_ps[:, :tw],
                                    scalar1=INV_D, scalar2=None, op0=ALU.mult)
            m2 = stat.tile([1, T], F32, tag="m2", bufs=1, name="m2")
            nc.vector.tensor_mul(out=m2[:, :tw], in0=mean[:, :tw],
                                 in1=mean[:, :tw])
            var = stat.tile([1, T], F32, tag="var", bufs=1, name="var")
            nc.vector.tensor_scalar(out=var[:, :tw], in0=ss_ps[:, :tw],
                                    scalar1=INV_D1, scalar2=None, op0=ALU.mult)
            nc.vector.tensor_scalar(out=m2[:, :tw], in0=m2[:, :tw],
                                    scalar1=D_OVER_D1, scalar2=None,
                                    op0=ALU.mult)
            nc.vector.tensor_tensor(var[:, :tw], var[:, :tw], m2[:, :tw],
                                    ALU.subtract)
            nc.scalar.activation(out=var[:, :tw], in_=var[:, :tw], func=AF.Sqrt)
            rstd = stat.tile([1, T], F32, tag="rstd", bufs=1, name="rstd")
            nc.vector.reciprocal(out=rstd[:, :tw], in_=var[:, :tw])
            nmr = stat.tile([1, T], F32, tag="nmr", bufs=1, name="nmr")
            nc.vector.tensor_mul(out=nmr[:, :tw], in0=mean[:, :tw],
                                 in1=rstd[:, :tw])
            nc.vector.tensor_scalar(out=nmr[:, :tw], in0=nmr[:, :tw],
                                    scalar1=-1.0, scalar2=None, op0=ALU.mult)
            rstd_ps = ps_tile()
            nmr_ps = ps_tile()
            for nh in range(0, tw, 512):
                w = min(512, tw - nh)
                nc.tensor.matmul(rstd_ps[:, nh:nh + w], ones_row[:],
                                 rstd[:, nh:nh + w], start=True, stop=True)
                nc.tensor.matmul(nmr_ps[:, nh:nh + w], ones_row[:],
                                 nmr[:, nh:nh + w], start=True, stop=True)
            rstd_bc = scr.tile([128, T], F32, tag="bc", bufs=2, name="rstd_bc")
            nc.vector.tensor_copy(out=rstd_bc[:, :tw], in_=rstd_ps[:, :tw])
            nmr_bc = scr.tile([128, T], F32, tag="bc", bufs=2, name="nmr_bc")
            nc.vector.tensor_copy(out=nmr_bc[:, :tw], in_=nmr_ps[:, :tw])
            a_src = z_sb if apply_src is None else apply_src
            for c in range(NCH):
                tmp = scr.tile([128, T], F32, tag="s4", bufs=4, name="ntmp")
                nc.vector.tensor_mul(out=tmp[:, :tw],
                                     in0=a_src[:, c, off:off + tw],
                                     in1=rstd_bc[:, :tw])
                if chunk_writer is None:
                    nc.vector.tensor_tensor(out_sb[:, c, off:off + tw], tmp[:, :tw],
                                            nmr_bc[:, :tw], ALU.add)
                else:
                    oc = scr.tile([128, T], F32, tag="s4", bufs=4, name="oc")
                    nc.vector.tensor_tensor(oc[:, :tw], tmp[:, :tw],
                                            nmr_bc[:, :tw], ALU.add)
                    chunk_writer(c, oc[:, :tw])

        # ================= attention inner block (scores/exp/AV) ===========
        # proj_spec: list of (dst, w_t, src, fold) projected per-pair right
        # before that pair's scores — keeps PE dense while ACT runs exps.
        def attn_inner(qt_sb, kt_sb, vv_sb, aot_sb, proj_spec=()):
            for p in range(NPAIR):
                for dst, w_t, src, fld in proj_spec:
                    pp = ps_tile()
                    for c in range(NCH):
                        lhsT = w_t[:, c, 128 * p:128 * (p + 1)]
                        for nh in range(2):
                            nc.tensor.matmul(
                                pp[:, 512 * nh:512 * (nh + 1)], lhsT,
                                src[:, c, 512 * nh:512 * (nh + 1)],
                                start=(c == 0), stop=(c == NCH - 1))
                    if fld is None:
                        nc.vector.tensor_copy(out=dst[:, p, :], in_=pp[:])
                    else:
                        nc.vector.tensor_scalar(
                            out=dst[:, p, :], in0=pp[:], scalar1=fld[p][:],
                            scalar2=None, op0=ALU.mult)
                av_ps = ps_tile()
                for st in range(NCH):
                    zp = stat.tile([128, 2], F32, tag="zp", bufs=4, name="zp")
                    e_pair = []
                    for h in range(2):
                        sc_ps = ps_tile()
                        k0 = 64 * h
                        lhsT = kt_sb[k0:k0 + 64, p, 128 * st:128 * (st + 1)]
                        for nh in range(2):
                            nc.tensor.matmul(
                                sc_ps[:, 512 * nh:512 * (nh + 1)], lhsT,
                                qt_sb[k0:k0 + 64, p, 512 * nh:512 * (nh + 1)],
                                start=True, stop=True, tile_position=(k0, 0))
                        e_st = epool.tile([128, T], BF16, tag="E", bufs=3,
                                          name="e_st")
                        nc.scalar.activation(
                            out=e_st[:], in_=sc_ps[:], func=AF.Exp, scale=ISQ,
                            accum_out=zp[:, h:h + 1])
                        e_pair.append(e_st)
                    rp = stat.tile([128, 2], F32, tag="rp", bufs=4, name="rp")
                    nc.vector.reciprocal(out=rp[:], in_=zp[:])
                    vv_sc = scr.tile([128, 2, 64], BF16, tag="vvsc", bufs=3,
                                     name="vv_sc")
                    nc.vector.tensor_tensor(
                        vv_sc[:],
                        vv_sb[:, st, 128 * p:128 * (p + 1)].rearrange(
                            "s (h v) -> s h v", h=2),
                        rp[:, :, None].to_broadcast([128, 2, 64]),
                        ALU.mult)
                    for h in range(2):
                        for nh in range(2):
                            nc.tensor.matmul(
                                av_ps[64 * h:64 * (h + 1),
                                      512 * nh:512 * (nh + 1)],
                                vv_sc[:, h, :],
                                e_pair[h][:, 512 * nh:512 * (nh + 1)],
                                start=(st == 0), stop=(st == NCH - 1),
                                tile_position=(0, 64 * h))
                nc.vector.tensor_copy(out=aot_sb[:, p, :], in_=av_ps[:])

        # ================= projection helpers ==============================
        def proj_qk(dst, w_t, src, fold):
            """dst[:, p, :] = (W_pair^T @ src) [optionally * fold[p] rows]."""
            for p in range(NPAIR):
                pp = ps_tile()
                for c in range(NCH):
                    lhsT = w_t[:, c, 128 * p:128 * (p + 1)]
                    for nh in range(2):
                        nc.tensor.matmul(
                            pp[:, 512 * nh:512 * (nh + 1)], lhsT,
                            src[:, c, 512 * nh:512 * (nh + 1)],
                            start=(c == 0), stop=(c == NCH - 1))
                if fold is None:
                    nc.vector.tensor_copy(out=dst[:, p, :], in_=pp[:])
                else:
                    nc.vector.tensor_scalar(
                        out=dst[:, p, :], in0=pp[:], scalar1=fold[p][:],
                        scalar2=None, op0=ALU.mult)

        def proj_v(dst, w_t, src):
            for st in range(NCH):
                vp = ps_tile((128, 512))
                for c in range(NCH):
                    nc.tensor.matmul(
                        vp[:], src[:, c, 128 * st:128 * (st + 1)],
                        w_t[:, c, :], start=(c == 0), stop=(c == NCH - 1))
                nc.vector.tensor_copy(out=dst[:, st, :], in_=vp[:])

        def wo_partial(wo_sb, aot_sb, emit_e_tile):
            for e in range(NCH):
                wo_ps = ps_tile()
                for p in range(NPAIR):
                    lhsT = wo_sb[:, p, 128 * e:128 * (e + 1)]
                    for nh in range(2):
                        nc.tensor.matmul(
                            wo_ps[:, 512 * nh:512 * (nh + 1)], lhsT,
                            aot_sb[:, p, 512 * nh:512 * (nh + 1)],
                            start=(p == 0), stop=(p == NPAIR - 1))
                emit_e_tile(e, wo_ps)

        def load_w(pool, name, tag):
            t = pool.tile([128, NCH, 512], BF16, tag=tag, name=name + "_sb")
            nc.sync.dma_start(t[:], io[name].ap().rearrange("c p k -> p c k"))
            return t

        # ============================ start =================================
        with tc.tile_pool(name="actA", bufs=1) as actA:
            y_sb = actA.tile([128, NCH, T], BF16, tag="y", name="y_sb")

            with tc.tile_pool(name="gio", bufs=1) as gio:
                qt = gio.tile([128, NPAIR, T], BF16, tag="qt", name="qt")
                kt = gio.tile([128, NPAIR, T], BF16, tag="kt", name="kt")
                vv = gio.tile([128, NCH, 512], BF16, tag="vv", name="vv")
                aot = gio.tile([128, NPAIR, T], BF16, tag="aot", name="aot")

                ag_in = dram.tile([NPAIR, 128, T], BF16, tag="ag_in",
                                  name="ag_in")
                ag_out = dram.tile([2, NPAIR, 128, T], BF16, tag="ag_out",
                                   name="ag_out")
                TQ = THALF // 2
                rs_inA = dram.tile([2, NCH, 128, TQ], BF16, tag="rs_inA",
                                   name="rs_inA")
                rs_inB = dram.tile([2, NCH, 128, TQ], BF16, tag="rs_inB",
                                   name="rs_inB")
                rs_outA = dram.tile([NCH, 128, TQ], BF16, tag="rs_outA",
                                    name="rs_outA")
                rs_outB = dram.tile([NCH, 128, TQ], BF16, tag="rs_outB",
                                    name="rs_outB")

                # ---------------- Layer 1 ----------------
                with tc.tile_pool(name="w1", bufs=1) as w1:
                    # masked weight softmax for Wq1/Wk1 (no max-subtraction)
                    ewq = w1.tile([128, NCH, 512], BF16, tag="ewq", name="ewq")
                    ewk = w1.tile([128, NCH, 512], BF16, tag="ewk", name="ewk")
                    for nm, ew in (("wq1", ewq), ("wk1", ewk)):
                        raw = w1.tile([128, NCH, 512], BF16, tag="wraw",
                                      bufs=1, name="wraw")
                        nc.sync.dma_start(
                            raw[:], io[nm].ap().rearrange("c p k -> p c k"))
                        nc.scalar.activation(out=ew[:], in_=raw[:], func=AF.Exp)
                        nc.vector.tensor_tensor(
                            ew[:, 0, :].rearrange("p (q k) -> p q k", q=NPAIR),
                            ew[:, 0, :].rearrange("p (q k) -> p q k", q=NPAIR),
                            mask_sb[:, None, :].to_broadcast([128, NPAIR, 128]),
                            ALU.mult)
                    nc.sync.dma_start(
                        y_sb[:], io["yT"].ap().rearrange("c p t -> p c t"))
                    # column sums over d -> ck = 1/(Sq*Sk), transposed per pair
                    sq_ps = ps_tile((1, 512))
                    for c in range(NCH):
                        nc.tensor.matmul(sq_ps[:], ones_col[:], ewq[:, c, :],
                                         start=(c == 0), stop=(c == NCH - 1))
                    sk_ps = ps_tile((1, 512))
                    for c in range(NCH):
                        nc.tensor.matmul(sk_ps[:], ones_col[:], ewk[:, c, :],
                                         start=(c == 0), stop=(c == NCH - 1))
                    rq = stat.tile([1, 512], F32, tag="rq", name="rq")
                    rk = stat.tile([1, 512], F32, tag="rk", name="rk")
                    nc.vector.reciprocal(out=rq[:], in_=sq_ps[:])
                    nc.vector.reciprocal(out=rk[:], in_=sk_ps[:])
                    ckk = stat.tile([1, 512], F32, tag="ck", name="ckk")
                    nc.vector.tensor_mul(out=ckk[:], in0=rq[:], in1=rk[:])
                    ckT = []
                    for p in range(NPAIR):
                        ct_ps = ps_tile((128, 1))
                        nc.tensor.matmul(ct_ps[:],
                                         ckk[:, 128 * p:128 * (p + 1)],
                                         one1[:], start=True, stop=True)
                        ct = stat.tile([128, 1], F32, tag=f"ckT{p}",
                                       name=f"ckT{p}")
                        nc.vector.tensor_copy(out=ct[:], in_=ct_ps[:])
                        ckT.append(ct)

                    wv1 = load_w(w1, "wv1", "wv")
                    proj_v(vv, wv1, y_sb)

                    proj_qk(qt, ewq, y_sb, None)
                    proj_qk(kt, ewk, y_sb, ckT)
                    attn_inner(qt, kt, vv, aot)
                    _dump(nc, dbg, "d_qt1", qt[:])
                    _dump(nc, dbg, "d_kt1", kt[:])
                    _dump(nc, dbg, "d_vv1", vv[:])
                    _dump(nc, dbg, "d_aot1", aot[:])
                    nc.sync.dma_start(ag_in.rearrange("q p t -> p q t"),
                                      aot[:])
                nc.gpsimd.collective_compute(
                    "AllGather", ALU.bypass, replica_groups=REPLICA_GROUPS,
                    ins=[ag_in.opt()], outs=[ag_out.opt()])

                # ---------------- Layer 2 (overlaps the AllReduce) ---------
                with tc.tile_pool(name="actB", bufs=1) as actB:
                    x_sb = actB.tile([128, NCH, T], BF16, tag="x", name="x_sb")
                    nc.sync.dma_start(
                        x_sb[:], io["xT"].ap().rearrange("c p t -> p c t"))
                    with tc.tile_pool(name="w2", bufs=1) as w2:
                        wq2 = load_w(w2, "wq2", "wq2")
                        wk2 = load_w(w2, "wk2", "wk2")
                        wv2 = load_w(w2, "wv2", "wv2")
                        wo2 = w2.tile([128, NPAIR, D], BF16, tag="wo2",
                                      name="wo2")
                        nc.sync.dma_start(
                            wo2[:], io["wo2"].ap().rearrange("q p e -> p q e"))
                        # K/V projections only need x -> run during the AR
                        proj_qk(kt, wk2, x_sb, None)
                        proj_v(vv, wv2, x_sb)

                        # gather both cores' AOT, full Wo1 locally;
                        # z1 = Wo1(aot_full) + y(fp32), in place into y_sb
                        wo1f = w2.tile([128, 2, NPAIR, D], BF16, tag="wo1f",
                                       name="wo1f")
                        nc.sync.dma_start(
                            wo1f[:], io["wo1"].ap()
                            .rearrange("(r q) p e -> p r q e", r=2))
                        aot_full = w2.tile([128, 2, NPAIR, T], BF16,
                                           tag="aotf", name="aot_full")
                        nc.sync.dma_start(
                            aot_full[:],
                            ag_out.rearrange("r q p t -> p r q t"))
                        for e in range(NCH):
                            wo_ps = ps_tile()
                            for r in range(2):
                                for p in range(NPAIR):
                                    lhsT = wo1f[:, r, p, 128 * e:128 * (e + 1)]
                                    for nh in range(2):
                                        nc.tensor.matmul(
                                            wo_ps[:, 512 * nh:512 * (nh + 1)],
                                            lhsT,
                                            aot_full[:, r, p,
                                                     512 * nh:512 * (nh + 1)],
                                            start=(r == 0 and p == 0),
                                            stop=(r == 1 and p == NPAIR - 1))
                            yf = scr.tile([128, T], F32, tag="s4", bufs=4,
                                          name="yf")
                            nc.sync.dma_start(yf[:], io["yT32"].ap()[e])
                            nc.vector.tensor_tensor(y_sb[:, e, :], wo_ps[:],
                                                    yf[:], ALU.add)
                        out1 = y_sb  # alias: z1 normalized in place
                        norm_into(y_sb, T, out1)
                        _dump(nc, dbg, "d_out1T", out1[:])

                        proj_qk(qt, wq2, out1, None)
                        attn_inner(qt, kt, vv, aot)

                        def emit_rs(e, wo_ps):
                            half = scr.tile([128, T], BF16, tag="sb4", bufs=4,
                                            name="half")
                            nc.vector.tensor_scalar(
                                out=half[:], in0=out1[:, e, :], scalar1=0.5,
                                scalar2=None, op0=ALU.mult)
                            res = scr.tile([128, T], BF16, tag="sb4", bufs=4,
                                           name="res")
                            nc.vector.tensor_tensor(res[:], wo_ps[:], half[:],
                                                    ALU.add)
                            nc.sync.dma_start(rs_inA[0, e], res[:, 0:TQ])
                            nc.sync.dma_start(rs_inA[1, e],
                                              res[:, THALF:THALF + TQ])
                            nc.sync.dma_start(rs_inB[0, e], res[:, TQ:THALF])
                            nc.sync.dma_start(rs_inB[1, e], res[:, THALF + TQ:])

                        wo_partial(wo2, aot, emit_rs)
                nc.gpsimd.collective_compute(
                    "ReduceScatter", ALU.add, replica_groups=REPLICA_GROUPS,
                    ins=[rs_inA.opt()], outs=[rs_outA.opt()])
                nc.gpsimd.collective_compute(
                    "ReduceScatter", ALU.add, replica_groups=REPLICA_GROUPS,
                    ins=[rs_inB.opt()], outs=[rs_outB.opt()])

            # ---------------- FFN on local t-half ----------------
            with tc.tile_pool(name="fh", bufs=1) as fh:
                h_sb = fh.tile([128, NFT, THALF], BF16, tag="h", name="h_sb")
                NWIN = 4
                pre_win = {}
                for ft in range(NWIN):
                    wt = fh.tile([128, NCH, 128], BF16, tag="win",
                                 bufs=NWIN, name="win_t")
                    nc.sync.dma_start(
                        wt[:], io["win"].ap()[ft].rearrange("c p f -> p c f"))
                    pre_win[ft] = wt
                pre_wout = fh.tile([128, NFT, 128], BF16, tag="wout", bufs=2,
                                   name="wout_t")
                nc.sync.dma_start(pre_wout[:],
                                  io["wout"].ap()[0].rearrange("f p m -> p f m"))

                TQ = THALF // 2
                z2 = hz.tile([128, NCH, THALF], BF16, tag="hz", bufs=1, name="z2")
                nc.sync.dma_start(z2[:, :, 0:TQ],
                                  rs_outA.rearrange("c p t -> p c t"))
                out2 = z2
                norm_into(z2, TQ, out2, off=0)
                nc.sync.dma_start(z2[:, :, TQ:THALF],
                                  rs_outB.rearrange("c p t -> p c t"))
                win_tiles = {}
                for ft in range(NFT):
                    if ft in pre_win:
                        win_t = pre_win.pop(ft)
                    else:
                        win_t = fh.tile([128, NCH, 128], BF16, tag="win",
                                        bufs=NWIN, name="win_t")
                        nc.sync.dma_start(
                            win_t[:],
                            io["win"].ap()[ft].rearrange("c p f -> p c f"))
                    win_tiles[ft] = win_t
                    hp = ps_tile((128, THALF))
                    for c in range(NCH):
                        nc.tensor.matmul(hp[:, 0:TQ], win_t[:, c, :],
                                         out2[:, c, 0:TQ],
                                         start=(c == 0), stop=(c == NCH - 1))
                    nc.scalar.activation(out=h_sb[:, ft, 0:TQ], in_=hp[:, 0:TQ],
                                         func=AF.Relu,
                                         bias=bin_sb[:, ft:ft + 1], scale=1.0)
                    if ft == 0:
                        norm_into(z2, TQ, out2, off=TQ)
                    hp2 = ps_tile((128, THALF))
                    for c in range(NCH):
                        nc.tensor.matmul(hp2[:, 0:TQ], win_t[:, c, :],
                                         out2[:, c, TQ:THALF],
                                         start=(c == 0), stop=(c == NCH - 1))
                    nc.scalar.activation(out=h_sb[:, ft, TQ:THALF],
                                         in_=hp2[:, 0:TQ], func=AF.Relu,
                                         bias=bin_sb[:, ft:ft + 1], scale=1.0)
                    del win_tiles[ft]

                z3 = out2  # in-place: out2[:, e] is last read by this add
                z3f = fh.tile([128, NCH, THALF], F32, tag="z3f", name="z3f")
                for e in range(NCH):
                    if e == 0:
                        wout_t = pre_wout
                    else:
                        wout_t = fh.tile([128, NFT, 128], BF16,
                                         tag="wout", bufs=2, name="wout_t")
                        nc.sync.dma_start(
                            wout_t[:],
                            io["wout"].ap()[e].rearrange("f p m -> p f m"))
                    fp = ps_tile((128, THALF))
                    for fc in range(NFT):
                        nc.tensor.matmul(
                            fp[:], wout_t[:, fc, :], h_sb[:, fc, :],
                            start=(fc == 0), stop=(fc == NFT - 1))
                    t1 = scr.tile([128, T], F32, tag="s4", bufs=4, name="fftmp")
                    nc.vector.tensor_scalar(out=t1[:, :THALF], in0=fp[:],
                                            scalar1=bout_sb[:, e:e + 1],
                                            scalar2=None, op0=ALU.add)
                    nc.vector.tensor_tensor(z3f[:, e, :], t1[:, :THALF],
                                            out2[:, e, :], ALU.add)
                    nc.vector.tensor_copy(out=z3[:, e, :], in_=z3f[:, e, :])
                norm_into(z3, THALF, None,
                          chunk_writer=lambda c, oc: nc.sync.dma_start(
                              out_d.ap()[c], oc), apply_src=z3f)


# ============================================================== host side ===
def _to_bf16(a):
    import ml_dtypes
    return np.asarray(a, np.float32).astype(ml_dtypes.bfloat16)


def _prep_inputs(inputs):
    """Per-core in_maps (host does transposes/tiling/dtype casts only)."""
    x = np.asarray(inputs["x"], np.float32)
    y = np.asarray(inputs["y"], np.float32)
    mask = _to_bf16(np.tile(np.tril(np.ones((128, DK), np.float32)), (1, 2)))
    win_t = _to_bf16(np.asarray(inputs["w_in"], np.float32).T
                     .reshape(NCH, 128, NFT, 128).transpose(2, 0, 1, 3))
    wout_t = _to_bf16(np.asarray(inputs["w_out"], np.float32).T
                      .reshape(NFT, 128, NCH, 128).transpose(2, 0, 1, 3))
    bin2 = np.ascontiguousarray(
        np.asarray(inputs["b_in"], np.float32).reshape(NFT, 128).T)
    bout2 = np.ascontiguousarray(
        np.asarray(inputs["b_out"], np.float32).reshape(NCH, 128).T)

    def packw(w, hs):
        return _to_bf16(np.asarray(w, np.float32)[hs].transpose(1, 0, 2)
                        .reshape(D, 512).reshape(NCH, 128, 512))

    shared = {"win": win_t, "wout": wout_t, "bin": bin2, "bout": bout2,
              "mask": mask}
    in_maps = []
    for c in range(NCORES):
        b, m = c // 2, c % 2
        hs = slice(8 * m, 8 * (m + 1))
        im = dict(shared)
        im["yT"] = _to_bf16(y[b].T.reshape(NCH, 128, T))
        im["yT32"] = np.ascontiguousarray(y[b].T.reshape(NCH, 128, T))
        im["xT"] = _to_bf16(x[b].T.reshape(NCH, 128, T))
        im["wq1"] = packw(inputs["Wq1"], hs)
        im["wk1"] = packw(inputs["Wk1"], hs)
        im["wv1"] = packw(inputs["Wv1"], hs)
        im["wq2"] = packw(inputs["Wq2"], hs)
        im["wk2"] = packw(inputs["Wk2"], hs)
        im["wv2"] = packw(inputs["Wv2"], hs)
        im["wo1"] = _to_bf16(np.asarray(inputs["Wo1"], np.float32)
                             .reshape(2 * NPAIR, 128, D))
        im["wo2"] = _to_bf16(np.asarray(inputs["Wo2"], np.float32)
                             [512 * m:512 * (m + 1)].reshape(NPAIR, 128, D))
        in_maps.append(im)
    return in_maps


def _assemble(results):
    out3 = np.empty((B, T, D), np.float32)
    for b in range(B):
        halves = [results[2 * b + m]["out3T"].reshape(D, THALF)
                  for m in range(2)]
        out3[b] = np.concatenate(halves, axis=1).T
    return out3


# ================================================================ runner ===
_CACHE = {}


def _make_runner(nc, n_cores):
    import jax
    from jax.sharding import Mesh, PartitionSpec
    from jax.experimental.shard_map import shard_map
    from concourse.bass2jax import (_bass_exec_p, install_neuronx_cc_hook,
                                    partition_id_tensor)

    install_neuronx_cc_hook()
    partition_name = nc.partition_id_tensor.name if nc.partition_id_tensor else None
    in_names, out_names, out_avals, zero_outs = [], [], [], []
    for alloc in nc.m.functions[0].allocations:
        if not isinstance(alloc, mybir.MemoryLocationSet):
            continue
        name = alloc.memorylocations[0].name
        if alloc.kind == "ExternalInput":
            if name != partition_name:
                in_names.append(name)
        elif alloc.kind == "ExternalOutput":
            shape = tuple(alloc.tensor_shape)
            dtype = mybir.dt.np(alloc.dtype)
            out_names.append(name)
            out_avals.append(jax.core.ShapedArray(shape, dtype))
            zero_outs.append(np.zeros(shape, dtype))
    n_params = len(in_names)
    n_outs = len(out_avals)
    all_in = in_names + out_names + ([partition_name] if partition_name else [])

    def _body(*args):
        operands = list(args)
        if partition_name is not None:
            operands.append(partition_id_tensor())
        return tuple(_bass_exec_p.bind(
            *operands, out_avals=tuple(out_avals), in_names=tuple(all_in),
            out_names=tuple(out_names), lowering_input_output_aliases=(),
            sim_require_finite=True, sim_require_nnan=True, nc=nc))

    devices = jax.devices()[:n_cores]
    mesh = Mesh(np.asarray(devices), ("core",))
    sharded = jax.jit(
        shard_map(_body, mesh=mesh,
                  in_specs=(PartitionSpec("core"),) * (n_params + n_outs),
                  out_specs=(PartitionSpec("core"),) * n_outs,
                  check_rep=False),
        keep_unused=True)

    def run(in_maps):
        concat_in = [
            np.concatenate([np.asarray(in_maps[c][nm]) for c in range(n_cores)],
                           axis=0)
            for nm in in_names
        ]
        concat_zero = [np.concatenate([z] * n_cores, axis=0) for z in zero_outs]
        outs = [np.asarray(o) for o in sharded(*concat_in, *concat_zero)]
        results = []
        for c in range(n_cores):
            r = {}
            for i, nm in enumerate(out_names):
                per = outs[i].shape[0] // n_cores
                r[nm] = outs[i][c * per:(c + 1) * per]
            results.append(r)
        return results

    return run


def _get_built(debug=False):
    key = "dbg" if debug else "main"
    if key not in _CACHE:
        nc = build_nc(debug=debug)
        run = _make_runner(nc, NCORES)
        _CACHE[key] = (nc, run)
    return _CACHE[key]


def kernel(**inputs):
    nc, run = _get_built()
    in_maps = _prep_inputs(inputs)
    results = run(in_maps)
    out3 = _assemble(results)
    return (np.asarray(inputs["x"], np.float32), out3)


# revision 30
# speedup vs baseline: 1.1072x; 1.0933x over previous
"""Trainium2 Bass kernel for nn_DecoderStack (2-layer decoder + FFN).

B=4 T=1024 D=1024 H=16 DK=DV=64 FF=4096, fp32 I/O.

Sharding (8 cores): core c -> batch b=c//2, head-group m=c%2 (8 of 16 heads).
Activations kept transposed on device: [d on partitions, t on free].
Matmul operands are bf16 (fp32 PSUM accumulation); stats/normalization in fp32.
Cross-core: per-pair AllReduce of the Wo1 partial; per-pair ReduceScatter of
(Wo2 partial + out1/2) giving each core its residual-included t-half for the
locally computed FFN.

kernel(**inputs) takes full unsharded inputs, returns (x, out3) like the ref.
"""
import contextlib

import numpy as np

import concourse.bass as bass  # noqa: F401
import concourse.tile as tile
from concourse import bacc, mybir

F32 = mybir.dt.float32
F32R = mybir.dt.float32r
BF16 = mybir.dt.bfloat16
AF = mybir.ActivationFunctionType
ALU = mybir.AluOpType

NCORES = 8
B, T, D, H, DK, DV, FF = 4, 1024, 1024, 16, 64, 64, 4096
NCH = D // 128          # 8 d-chunks of 128
NPAIR = 4               # head-pairs per core (8 heads)
THALF = T // 2
NFT = FF // 128         # 32 f-tiles
ISQ = float(1.0 / np.sqrt(np.float32(DK)))
INV_D = float(1.0 / D)
INV_D1 = float(1.0 / (D - 1))
D_OVER_D1 = float(D / (D - 1))

REPLICA_GROUPS = [[0, 1], [2, 3], [4, 5], [6, 7]]


# ================================================================ builder ===
def build_nc(debug=False):
    nc = bacc.Bacc("TRN2", target_bir_lowering=False, debug=False,
                   num_devices=NCORES)

    io = {}

    def din(name, shape, dt):
        io[name] = nc.dram_tensor(name, shape, dt, kind="ExternalInput")

    din("yT", [NCH, 128, T], BF16)
    din("yT32", [NCH, 128, T], F32)
    din("xT", [NCH, 128, T], BF16)
    din("wq1", [NCH, 128, 512], BF16)
    din("wk1", [NCH, 128, 512], BF16)
    din("wv1", [NCH, 128, 512], BF16)
    din("wo1", [2 * NPAIR, 128, D], BF16)
    din("wq2", [NCH, 128, 512], BF16)
    din("wk2", [NCH, 128, 512], BF16)
    din("wv2", [NCH, 128, 512], BF16)
    din("wo2", [NPAIR, 128, D], BF16)
    din("win", [NFT, NCH, 128, 128], BF16)
    din("wout", [NCH, NFT, 128, 128], BF16)
    din("bin", [128, NFT], F32)
    din("bout", [128, NCH], F32)
    din("mask", [128, 128], BF16)

    out_d = nc.dram_tensor("out3T", [NCH, 128, THALF], F32, kind="ExternalOutput")
    dbg = {}
    if debug:
        for name, shape, dt in (
                ("d_out1T", [128, NCH, T], BF16),
                ("d_qt1", [128, NPAIR, T], BF16),
                ("d_kt1", [128, NPAIR, T], BF16),
                ("d_vv1", [128, NCH, 512], BF16),
                ("d_aot1", [128, NPAIR, T], BF16),
                ("d_out2T", [128, NCH, THALF], BF16),
                ("d_z2", [128, NCH, THALF], BF16)):
            dbg[name] = nc.dram_tensor(name, shape, dt, kind="ExternalOutput")

    with tile.TileContext(nc) as tc:
        _emit(nc, tc, io, out_d, dbg)
    nc.compile()
    return nc


def _dump(nc, dbg, name, t_sb):
    if name in dbg:
        # stage through fp32 copy in DRAM-compatible layout
        nc.sync.dma_start(dbg[name].ap(), t_sb)


def _emit(nc, tc, io, out_d, dbg):
    ctx = contextlib.ExitStack()
    with ctx:
        # ---------------- outer pools (live whole kernel) ----------------
        const = ctx.enter_context(tc.tile_pool(name="const", bufs=1))
        stat = ctx.enter_context(tc.tile_pool(name="stat", bufs=1))
        scr = ctx.enter_context(tc.tile_pool(name="scr", bufs=1))
        epool = ctx.enter_context(tc.tile_pool(name="epool", bufs=1))
        hz = ctx.enter_context(tc.tile_pool(name="hz", bufs=1))
        dram = ctx.enter_context(tc.tile_pool(name="dram", bufs=1, space="DRAM"))
        psp = ctx.enter_context(tc.tile_pool(name="psp", bufs=1, space="PSUM"))

        def ps_tile(shape=(128, T)):
            return psp.tile(list(shape), F32, tag="ps", bufs=4, name="ps")

        # ---------------- constants ----------------
        ones_col = const.tile([128, 1], BF16)
        nc.vector.memset(ones_col[:], 1.0)
        ones_row = const.tile([1, 128], F32)
        nc.vector.memset(ones_row[:], 1.0)
        one1 = const.tile([1, 1], F32)
        nc.vector.memset(one1[:], 1.0)
        mask_sb = const.tile([128, 128], BF16)
        nc.sync.dma_start(mask_sb[:], io["mask"].ap())
        bin_sb = const.tile([128, NFT], F32)
        nc.sync.dma_start(bin_sb[:], io["bin"].ap())
        bout_sb = const.tile([128, NCH], F32)
        nc.sync.dma_start(bout_sb[:], io["bout"].ap())

        # =========== transposed-space layer norm (over d = partitions) =====
        def norm_into(z_sb, tw, out_sb, chunk_writer=None, off=0,
                      apply_src=None):
            """out_sb[:, c, :tw] = (z - mean)/std per t-column. z_sb bf16
            [128, NCH, tw]; out_sb may alias z_sb (chunk-wise in-place).
            If chunk_writer is given, per-chunk f32 results are handed to it
            instead of writing out_sb."""
            s_ps = ps_tile((1, T))
            for c in range(NCH):
                for nh in range(0, tw, 512):
                    w = min(512, tw - nh)
                    nc.tensor.matmul(
                        s_ps[:, nh:nh + w], ones_col[:],
                        z_sb[:, c, off + nh:off + nh + w],
                        start=(c == 0), stop=(c == NCH - 1))
            ss_ps = ps_tile((1, T))
            for c in range(NCH):
                zsq = scr.tile([128, T], BF16, tag="zsq", bufs=2, name="zsq")
                nc.vector.tensor_mul(out=zsq[:, :tw],
                                     in0=z_sb[:, c, off:off + tw],
                                     in1=z_sb[:, c, off:off + tw])
                for nh in range(0, tw, 512):
                    w = min(512, tw - nh)
                    nc.tensor.matmul(
                        ss_ps[:, nh:nh + w], ones_col[:], zsq[:, nh:nh + w],
                        start=(c == 0), stop=(c == NCH - 1))
            mean = stat.tile([1, T], F32, tag="mean", bufs=1, name="mean")
            nc.vector.tensor_scalar(out=mean[:, :tw], in0=s_ps[:, :tw],
                                    scalar1=INV_D, scalar2=None, op0=ALU.mult)
            m2 = stat.tile([1, T], F32, tag="m2", bufs=1, name="m2")
            nc.vector.tensor_mul(out=m2[:, :tw], in0=mean[:, :tw],
                                 in1=mean[:, :tw])
            var = stat.tile([1, T], F32, tag="var", bufs=1, name="var")
            nc.vector.tensor_scalar(out=var[:, :tw], in0=ss_ps[:, :tw],
                                    scalar1=INV_D1, scalar2=None, op0=ALU.mult)
            nc.vector.tensor_scalar(out=m2[:, :tw], in0=m2[:, :tw],
                                    scalar1=D_OVER_D1, scalar2=None,
                                    op0=ALU.mult)
            nc.vector.tensor_tensor(var[:, :tw], var[:, :tw], m2[:, :tw],
                                    ALU.subtract)
            nc.scalar.activation(out=var[:, :tw], in_=var[:, :tw], func=AF.Sqrt)
            rstd = stat.tile([1, T], F32, tag="rstd", bufs=1, name="rstd")
            nc.vector.reciprocal(out=rstd[:, :tw], in_=var[:, :tw])
            nmr = stat.tile([1, T], F32, tag="nmr", bufs=1, name="nmr")
            nc.vector.tensor_mul(out=nmr[:, :tw], in0=mean[:, :tw],
                                 in1=rstd[:, :tw])
            nc.vector.tensor_scalar(out=nmr[:, :tw], in0=nmr[:, :tw],
                                    scalar1=-1.0, scalar2=None, op0=ALU.mult)
            rstd_ps = ps_tile()
            nmr_ps = ps_tile()
            for nh in range(0, tw, 512):
                w = min(512, tw - nh)
                nc.tensor.matmul(rstd_ps[:, nh:nh + w], ones_row[:],
                                 rstd[:, nh:nh + w], start=True, stop=True)
                nc.tensor.matmul(nmr_ps[:, nh:nh + w], ones_row[:],
                                 nmr[:, nh:nh + w], start=True, stop=True)
            rstd_bc = scr.tile([128, T], F32, tag="bc", bufs=2, name="rstd_bc")
            nc.vector.tensor_copy(out=rstd_bc[:, :tw], in_=rstd_ps[:, :tw])
            nmr_bc = scr.tile([128, T], F32, tag="bc", bufs=2, name="nmr_bc")
            nc.vector.tensor_copy(out=nmr_bc[:, :tw], in_=nmr_ps[:, :tw])
            a_src = z_sb if apply_src is None else apply_src
            for c in range(NCH):
                tmp = scr.tile([128, T], F32, tag="s4", bufs=4, name="ntmp")
                nc.vector.tensor_mul(out=tmp[:, :tw],
                                     in0=a_src[:, c, off:off + tw],
                                     in1=rstd_bc[:, :tw])
                if chunk_writer is None:
                    nc.vector.tensor_tensor(out_sb[:, c, off:off + tw], tmp[:, :tw],
                                            nmr_bc[:, :tw], ALU.add)
                else:
                    oc = scr.tile([128, T], F32, tag="s4", bufs=4, name="oc")
                    nc.vector.tensor_tensor(oc[:, :tw], tmp[:, :tw],
                                            nmr_bc[:, :tw], ALU.add)
                    chunk_writer(c, oc[:, :tw])

        # ================= attention inner block (scores/exp/AV) ===========
        # proj_spec: list of (dst, w_t, src, fold) projected per-pair right
        # before that pair's scores — keeps PE dense while ACT runs exps.
        def attn_inner(qt_sb, kt_sb, vv_sb, aot_sb, proj_spec=()):
            for p in range(NPAIR):
                for dst, w_t, src, fld in proj_spec:
                    pp = ps_tile()
                    for c in range(NCH):
                        lhsT = w_t[:, c, 128 * p:128 * (p + 1)]
                        for nh in range(2):
                            nc.tensor.matmul(
                                pp[:, 512 * nh:512 * (nh + 1)], lhsT,
                                src[:, c, 512 * nh:512 * (nh + 1)],
                                start=(c == 0), stop=(c == NCH - 1))
                    if fld is None:
                        nc.vector.tensor_copy(out=dst[:, p, :], in_=pp[:])
                    else:
                        nc.vector.tensor_scalar(
                            out=dst[:, p, :], in0=pp[:], scalar1=fld[p][:],
                            scalar2=None, op0=ALU.mult)
                av_ps = ps_tile()

                def emit_av(st, e_pair, zp):
                    rp = stat.tile([128, 2], F32, tag="rp", bufs=4, name="rp")
                    nc.vector.reciprocal(out=rp[:], in_=zp[:])
                    vv_sc = scr.tile([128, 2, 64], BF16, tag="vvsc", bufs=3,
                                     name="vv_sc")
                    nc.vector.tensor_tensor(
                        vv_sc[:],
                        vv_sb[:, st, 128 * p:128 * (p + 1)].rearrange(
                            "s (h v) -> s h v", h=2),
                        rp[:, :, None].to_broadcast([128, 2, 64]),
                        ALU.mult)
                    for h in range(2):
                        for nh in range(2):
                            nc.tensor.matmul(
                                av_ps[64 * h:64 * (h + 1),
                                      512 * nh:512 * (nh + 1)],
                                vv_sc[:, h, :],
                                e_pair[h][:, 512 * nh:512 * (nh + 1)],
                                start=(st == 0), stop=(st == NCH - 1),
                                tile_position=(0, 64 * h))

                prev = None  # one-step software pipeline: scores(st+1) issue
                for st in range(NCH):  # before AV(st)'s exp-gated wait
                    zp = stat.tile([128, 2], F32, tag="zp", bufs=4, name="zp")
                    e_pair = []
                    for h in range(2):
                        sc_ps = ps_tile()
                        k0 = 64 * h
                        lhsT = kt_sb[k0:k0 + 64, p, 128 * st:128 * (st + 1)]
                        for nh in range(2):
                            nc.tensor.matmul(
                                sc_ps[:, 512 * nh:512 * (nh + 1)], lhsT,
                                qt_sb[k0:k0 + 64, p, 512 * nh:512 * (nh + 1)],
                                start=True, stop=True, tile_position=(k0, 0))
                        e_st = epool.tile([128, T], BF16, tag="E", bufs=4,
                                          name="e_st")
                        nc.scalar.activation(
                            out=e_st[:], in_=sc_ps[:], func=AF.Exp, scale=ISQ,
                            accum_out=zp[:, h:h + 1])
                        e_pair.append(e_st)
                    if prev is not None:
                        emit_av(*prev)
                    prev = (st, e_pair, zp)
                emit_av(*prev)
                nc.vector.tensor_copy(out=aot_sb[:, p, :], in_=av_ps[:])

        # ================= projection helpers ==============================
        def proj_qk(dst, w_t, src, fold):
            """dst[:, p, :] = (W_pair^T @ src) [optionally * fold[p] rows]."""
            for p in range(NPAIR):
                pp = ps_tile()
                for c in range(NCH):
                    lhsT = w_t[:, c, 128 * p:128 * (p + 1)]
                    for nh in range(2):
                        nc.tensor.matmul(
                            pp[:, 512 * nh:512 * (nh + 1)], lhsT,
                            src[:, c, 512 * nh:512 * (nh + 1)],
                            start=(c == 0), stop=(c == NCH - 1))
                if fold is None:
                    nc.vector.tensor_copy(out=dst[:, p, :], in_=pp[:])
                else:
                    nc.vector.tensor_scalar(
                        out=dst[:, p, :], in0=pp[:], scalar1=fold[p][:],
                        scalar2=None, op0=ALU.mult)

        def proj_v(dst, w_t, src):
            for st in range(NCH):
                vp = ps_tile((128, 512))
                for c in range(NCH):
                    nc.tensor.matmul(
                        vp[:], src[:, c, 128 * st:128 * (st + 1)],
                        w_t[:, c, :], start=(c == 0), stop=(c == NCH - 1))
                nc.vector.tensor_copy(out=dst[:, st, :], in_=vp[:])

        def wo_partial(wo_sb, aot_sb, emit_e_tile):
            for e in range(NCH):
                wo_ps = ps_tile()
                for p in range(NPAIR):
                    lhsT = wo_sb[:, p, 128 * e:128 * (e + 1)]
                    for nh in range(2):
                        nc.tensor.matmul(
                            wo_ps[:, 512 * nh:512 * (nh + 1)], lhsT,
                            aot_sb[:, p, 512 * nh:512 * (nh + 1)],
                            start=(p == 0), stop=(p == NPAIR - 1))
                emit_e_tile(e, wo_ps)

        def load_w(pool, name, tag):
            t = pool.tile([128, NCH, 512], BF16, tag=tag, name=name + "_sb")
            nc.sync.dma_start(t[:], io[name].ap().rearrange("c p k -> p c k"))
            return t

        # ============================ start =================================
        with tc.tile_pool(name="actA", bufs=1) as actA:
            y_sb = actA.tile([128, NCH, T], BF16, tag="y", name="y_sb")

            with tc.tile_pool(name="gio", bufs=1) as gio:
                qt = gio.tile([128, NPAIR, T], BF16, tag="qt", name="qt")
                kt = gio.tile([128, NPAIR, T], BF16, tag="kt", name="kt")
                vv = gio.tile([128, NCH, 512], BF16, tag="vv", name="vv")
                aot = gio.tile([128, NPAIR, T], BF16, tag="aot", name="aot")

                ag_in = dram.tile([NPAIR, 128, T], BF16, tag="ag_in",
                                  name="ag_in")
                ag_out = dram.tile([2, NPAIR, 128, T], BF16, tag="ag_out",
                                   name="ag_out")
                TQ = THALF // 2
                rs_inA = dram.tile([2, NCH, 128, TQ], BF16, tag="rs_inA",
                                   name="rs_inA")
                rs_inB = dram.tile([2, NCH, 128, TQ], BF16, tag="rs_inB",
                                   name="rs_inB")
                rs_outA = dram.tile([NCH, 128, TQ], BF16, tag="rs_outA",
                                    name="rs_outA")
                rs_outB = dram.tile([NCH, 128, TQ], BF16, tag="rs_outB",
                                    name="rs_outB")

                # ---------------- Layer 1 ----------------
                with tc.tile_pool(name="w1", bufs=1) as w1:
                    # masked weight softmax for Wq1/Wk1 (no max-subtraction)
                    ewq = w1.tile([128, NCH, 512], BF16, tag="ewq", name="ewq")
                    ewk = w1.tile([128, NCH, 512], BF16, tag="ewk", name="ewk")
                    for nm, ew in (("wq1", ewq), ("wk1", ewk)):
                        raw = w1.tile([128, NCH, 512], BF16, tag="wraw",
                                      bufs=1, name="wraw")
                        nc.sync.dma_start(
                            raw[:], io[nm].ap().rearrange("c p k -> p c k"))
                        nc.scalar.activation(out=ew[:], in_=raw[:], func=AF.Exp)
                        nc.vector.tensor_tensor(
                            ew[:, 0, :].rearrange("p (q k) -> p q k", q=NPAIR),
                            ew[:, 0, :].rearrange("p (q k) -> p q k", q=NPAIR),
                            mask_sb[:, None, :].to_broadcast([128, NPAIR, 128]),
                            ALU.mult)
                    nc.sync.dma_start(
                        y_sb[:], io["yT"].ap().rearrange("c p t -> p c t"))
                    # column sums over d -> ck = 1/(Sq*Sk), transposed per pair
                    sq_ps = ps_tile((1, 512))
                    for c in range(NCH):
                        nc.tensor.matmul(sq_ps[:], ones_col[:], ewq[:, c, :],
                                         start=(c == 0), stop=(c == NCH - 1))
                    sk_ps = ps_tile((1, 512))
                    for c in range(NCH):
                        nc.tensor.matmul(sk_ps[:], ones_col[:], ewk[:, c, :],
                                         start=(c == 0), stop=(c == NCH - 1))
                    rq = stat.tile([1, 512], F32, tag="rq", name="rq")
                    rk = stat.tile([1, 512], F32, tag="rk", name="rk")
                    nc.vector.reciprocal(out=rq[:], in_=sq_ps[:])
                    nc.vector.reciprocal(out=rk[:], in_=sk_ps[:])
                    ckk = stat.tile([1, 512], F32, tag="ck", name="ckk")
                    nc.vector.tensor_mul(out=ckk[:], in0=rq[:], in1=rk[:])
                    ckT = []
                    for p in range(NPAIR):
                        ct_ps = ps_tile((128, 1))
                        nc.tensor.matmul(ct_ps[:],
                                         ckk[:, 128 * p:128 * (p + 1)],
                                         one1[:], start=True, stop=True)
                        ct = stat.tile([128, 1], F32, tag=f"ckT{p}",
                                       name=f"ckT{p}")
                        nc.vector.tensor_copy(out=ct[:], in_=ct_ps[:])
                        ckT.append(ct)

                    wv1 = load_w(w1, "wv1", "wv")
                    proj_v(vv, wv1, y_sb)

                    proj_qk(qt, ewq, y_sb, None)
                    proj_qk(kt, ewk, y_sb, ckT)
                    attn_inner(qt, kt, vv, aot)
                    _dump(nc, dbg, "d_qt1", qt[:])
                    _dump(nc, dbg, "d_kt1", kt[:])
                    _dump(nc, dbg, "d_vv1", vv[:])
                    _dump(nc, dbg, "d_aot1", aot[:])
                    nc.sync.dma_start(ag_in.rearrange("q p t -> p q t"),
                                      aot[:])
                nc.gpsimd.collective_compute(
                    "AllGather", ALU.bypass, replica_groups=REPLICA_GROUPS,
                    ins=[ag_in.opt()], outs=[ag_out.opt()])

                # ---------------- Layer 2 (overlaps the AllReduce) ---------
                with tc.tile_pool(name="actB", bufs=1) as actB:
                    x_sb = actB.tile([128, NCH, T], BF16, tag="x", name="x_sb")
                    nc.sync.dma_start(
                        x_sb[:], io["xT"].ap().rearrange("c p t -> p c t"))
                    with tc.tile_pool(name="w2", bufs=1) as w2:
                        wq2 = load_w(w2, "wq2", "wq2")
                        wk2 = load_w(w2, "wk2", "wk2")
                        wv2 = load_w(w2, "wv2", "wv2")
                        wo2 = w2.tile([128, NPAIR, D], BF16, tag="wo2",
                                      name="wo2")
                        nc.sync.dma_start(
                            wo2[:], io["wo2"].ap().rearrange("q p e -> p q e"))
                        # K/V projections only need x -> run during the AR
                        proj_qk(kt, wk2, x_sb, None)
                        proj_v(vv, wv2, x_sb)

                        # gather both cores' AOT, full Wo1 locally;
                        # z1 = Wo1(aot_full) + y(fp32), in place into y_sb
                        wo1f = w2.tile([128, 2, NPAIR, D], BF16, tag="wo1f",
                                       name="wo1f")
                        nc.sync.dma_start(
                            wo1f[:], io["wo1"].ap()
                            .rearrange("(r q) p e -> p r q e", r=2))
                        aot_full = w2.tile([128, 2, NPAIR, T], BF16,
                                           tag="aotf", name="aot_full")
                        nc.sync.dma_start(
                            aot_full[:],
                            ag_out.rearrange("r q p t -> p r q t"))
                        for e in range(NCH):
                            wo_ps = ps_tile()
                            for r in range(2):
                                for p in range(NPAIR):
                                    lhsT = wo1f[:, r, p, 128 * e:128 * (e + 1)]
                                    for nh in range(2):
                                        nc.tensor.matmul(
                                            wo_ps[:, 512 * nh:512 * (nh + 1)],
                                            lhsT,
                                            aot_full[:, r, p,
                                                     512 * nh:512 * (nh + 1)],
                                            start=(r == 0 and p == 0),
                                            stop=(r == 1 and p == NPAIR - 1))
                            yf = scr.tile([128, T], F32, tag="s4", bufs=4,
                                          name="yf")
                            nc.sync.dma_start(yf[:], io["yT32"].ap()[e])
                            nc.vector.tensor_tensor(y_sb[:, e, :], wo_ps[:],
                                                    yf[:], ALU.add)
                        out1 = y_sb  # alias: z1 normalized in place
                        norm_into(y_sb, T, out1)
                        _dump(nc, dbg, "d_out1T", out1[:])

                        proj_qk(qt, wq2, out1, None)
                        attn_inner(qt, kt, vv, aot)

                        def emit_rs(e, wo_ps):
                            half = scr.tile([128, T], BF16, tag="sb4", bufs=2,
                                            name="half")
                            nc.vector.tensor_scalar(
                                out=half[:], in0=out1[:, e, :], scalar1=0.5,
                                scalar2=None, op0=ALU.mult)
                            res = scr.tile([128, T], BF16, tag="sb4", bufs=2,
                                           name="res")
                            nc.vector.tensor_tensor(res[:], wo_ps[:], half[:],
                                                    ALU.add)
                            nc.sync.dma_start(rs_inA[0, e], res[:, 0:TQ])
                            nc.sync.dma_start(rs_inA[1, e],
                                              res[:, THALF:THALF + TQ])
                            nc.sync.dma_start(rs_inB[0, e], res[:, TQ:THALF])
                            nc.sync.dma_start(rs_inB[1, e], res[:, THALF + TQ:])

                        wo_partial(wo2, aot, emit_rs)
                nc.gpsimd.collective_compute(
                    "ReduceScatter", ALU.add, replica_groups=REPLICA_GROUPS,
                    ins=[rs_inA.opt()], outs=[rs_outA.opt()])
                nc.gpsimd.collective_compute(
                    "ReduceScatter", ALU.add, replica_groups=REPLICA_GROUPS,
                    ins=[rs_inB.opt()], outs=[rs_outB.opt()])

            # ---------------- FFN on local t-half ----------------
            with tc.tile_pool(name="fh", bufs=1) as fh:
                h_sb = fh.tile([128, NFT, THALF], BF16, tag="h", name="h_sb")
                NWIN = 4
                pre_win = {}
                for ft in range(NWIN):
                    wt = fh.tile([128, NCH, 128], BF16, tag="win",
                                 bufs=NWIN, name="win_t")
                    nc.sync.dma_start(
                        wt[:], io["win"].ap()[ft].rearrange("c p f -> p c f"))
                    pre_win[ft] = wt
                pre_wout = fh.tile([128, NFT, 128], BF16, tag="wout", bufs=2,
                                   name="wout_t")
                nc.sync.dma_start(pre_wout[:],
                                  io["wout"].ap()[0].rearrange("f p m -> p f m"))

                TQ = THALF // 2
                z2 = hz.tile([128, NCH, THALF], BF16, tag="hz", bufs=1, name="z2")
                nc.sync.dma_start(z2[:, :, 0:TQ],
                                  rs_outA.rearrange("c p t -> p c t"))
                out2 = z2
                norm_into(z2, TQ, out2, off=0)
                nc.sync.dma_start(z2[:, :, TQ:THALF],
                                  rs_outB.rearrange("c p t -> p c t"))
                win_tiles = {}
                for ft in range(NFT):
                    if ft in pre_win:
                        win_t = pre_win.pop(ft)
                    else:
                        win_t = fh.tile([128, NCH, 128], BF16, tag="win",
                                        bufs=NWIN, name="win_t")
                        nc.sync.dma_start(
                            win_t[:],
                            io["win"].ap()[ft].rearrange("c p f -> p c f"))
                    win_tiles[ft] = win_t
                    hp = ps_tile((128, THALF))
                    for c in range(NCH):
                        nc.tensor.matmul(hp[:, 0:TQ], win_t[:, c, :],
                                         out2[:, c, 0:TQ],
                                         start=(c == 0), stop=(c == NCH - 1))
                    nc.scalar.activation(out=h_sb[:, ft, 0:TQ], in_=hp[:, 0:TQ],
                                         func=AF.Relu,
                                         bias=bin_sb[:, ft:ft + 1], scale=1.0)
                    if ft == 0:
                        norm_into(z2, TQ, out2, off=TQ)
                    hp2 = ps_tile((128, THALF))
                    for c in range(NCH):
                        nc.tensor.matmul(hp2[:, 0:TQ], win_t[:, c, :],
                                         out2[:, c, TQ:THALF],
                                         start=(c == 0), stop=(c == NCH - 1))
                    nc.scalar.activation(out=h_sb[:, ft, TQ:THALF],
                                         in_=hp2[:, 0:TQ], func=AF.Relu,
                                         bias=bin_sb[:, ft:ft + 1], scale=1.0)
                    del win_tiles[ft]

                z3 = out2  # in-place: out2[:, e] is last read by this add
                z3f = fh.tile([128, NCH, THALF], F32, tag="z3f", name="z3f")
                for e in range(NCH):
                    if e == 0:
                        wout_t = pre_wout
                    else:
                        wout_t = fh.tile([128, NFT, 128], BF16,
                                         tag="wout", bufs=2, name="wout_t")
                        nc.sync.dma_start(
                            wout_t[:],
                            io["wout"].ap()[e].rearrange("f p m -> p f m"))
                    fp = ps_tile((128, THALF))
                    for fc in range(NFT):
                        nc.tensor.matmul(
                            fp[:], wout_t[:, fc, :], h_sb[:, fc, :],
                            start=(fc == 0), stop=(fc == NFT - 1))
                    t1 = scr.tile([128, T], F32, tag="s4", bufs=4, name="fftmp")
                    nc.vector.tensor_scalar(out=t1[:, :THALF], in0=fp[:],
                                            scalar1=bout_sb[:, e:e + 1],
                                            scalar2=None, op0=ALU.add)
                    nc.vector.tensor_tensor(z3f[:, e, :], t1[:, :THALF],
                                            out2[:, e, :], ALU.add)
                    nc.vector.tensor_copy(out=z3[:, e, :], in_=z3f[:, e, :])
                norm_into(z3, THALF, None,
                          chunk_writer=lambda c, oc: nc.sync.dma_start(
                              out_d.ap()[c], oc), apply_src=z3f)


# ============================================================== host side ===
def _to_bf16(a):
    import ml_dtypes
    return np.asarray(a, np.float32).astype(ml_dtypes.bfloat16)


def _prep_inputs(inputs):
    """Per-core in_maps (host does transposes/tiling/dtype casts only)."""
    x = np.asarray(inputs["x"], np.float32)
    y = np.asarray(inputs["y"], np.float32)
    mask = _to_bf16(np.tile(np.tril(np.ones((128, DK), np.float32)), (1, 2)))
    win_t = _to_bf16(np.asarray(inputs["w_in"], np.float32).T
                     .reshape(NCH, 128, NFT, 128).transpose(2, 0, 1, 3))
    wout_t = _to_bf16(np.asarray(inputs["w_out"], np.float32).T
                      .reshape(NFT, 128, NCH, 128).transpose(2, 0, 1, 3))
    bin2 = np.ascontiguousarray(
        np.asarray(inputs["b_in"], np.float32).reshape(NFT, 128).T)
    bout2 = np.ascontiguousarray(
        np.asarray(inputs["b_out"], np.float32).reshape(NCH, 128).T)

    def packw(w, hs):
        return _to_bf16(np.asarray(w, np.float32)[hs].transpose(1, 0, 2)
                        .reshape(D, 512).reshape(NCH, 128, 512))

    shared = {"win": win_t, "wout": wout_t, "bin": bin2, "bout": bout2,
              "mask": mask}
    in_maps = []
    for c in range(NCORES):
        b, m = c // 2, c % 2
        hs = slice(8 * m, 8 * (m + 1))
        im = dict(shared)
        im["yT"] = _to_bf16(y[b].T.reshape(NCH, 128, T))
        im["yT32"] = np.ascontiguousarray(y[b].T.reshape(NCH, 128, T))
        im["xT"] = _to_bf16(x[b].T.reshape(NCH, 128, T))
        im["wq1"] = packw(inputs["Wq1"], hs)
        im["wk1"] = packw(inputs["Wk1"], hs)
        im["wv1"] = packw(inputs["Wv1"], hs)
        im["wq2"] = packw(inputs["Wq2"], hs)
        im["wk2"] = packw(inputs["Wk2"], hs)
        im["wv2"] = packw(inputs["Wv2"], hs)
        im["wo1"] = _to_bf16(np.asarray(inputs["Wo1"], np.float32)
                             .reshape(2 * NPAIR, 128, D))
        im["wo2"] = _to_bf16(np.asarray(inputs["Wo2"], np.float32)
                             [512 * m:512 * (m + 1)].reshape(NPAIR, 128, D))
        in_maps.append(im)
    return in_maps


def _assemble(results):
    out3 = np.empty((B, T, D), np.float32)
    for b in range(B):
        halves = [results[2 * b + m]["out3T"].reshape(D, THALF)
                  for m in range(2)]
        out3[b] = np.concatenate(halves, axis=1).T
    return out3


# ================================================================ runner ===
_CACHE = {}


def _make_runner(nc, n_cores):
    import jax
    from jax.sharding import Mesh, PartitionSpec
    from jax.experimental.shard_map import shard_map
    from concourse.bass2jax import (_bass_exec_p, install_neuronx_cc_hook,
                                    partition_id_tensor)

    install_neuronx_cc_hook()
    partition_name = nc.partition_id_tensor.name if nc.partition_id_tensor else None
    in_names, out_names, out_avals, zero_outs = [], [], [], []
    for alloc in nc.m.functions[0].allocations:
        if not isinstance(alloc, mybir.MemoryLocationSet):
            continue
        name = alloc.memorylocations[0].name
        if alloc.kind == "ExternalInput":
            if name != partition_name:
                in_names.append(name)
        elif alloc.kind == "ExternalOutput":
            shape = tuple(alloc.tensor_shape)
            dtype = mybir.dt.np(alloc.dtype)
            out_names.append(name)
            out_avals.append(jax.core.ShapedArray(shape, dtype))
            zero_outs.append(np.zeros(shape, dtype))
    n_params = len(in_names)
    n_outs = len(out_avals)
    all_in = in_names + out_names + ([partition_name] if partition_name else [])

    def _body(*args):
        operands = list(args)
        if partition_name is not None:
            operands.append(partition_id_tensor())
        return tuple(_bass_exec_p.bind(
            *operands, out_avals=tuple(out_avals), in_names=tuple(all_in),
            out_names=tuple(out_names), lowering_input_output_aliases=(),
            sim_require_finite=True, sim_require_nnan=True, nc=nc))

    devices = jax.devices()[:n_cores]
    mesh = Mesh(np.asarray(devices), ("core",))
    sharded = jax.jit(
        shard_map(_body, mesh=mesh,
                  in_specs=(PartitionSpec("core"),) * (n_params + n_outs),
                  out_specs=(PartitionSpec("core"),) * n_outs,
                  check_rep=False),
        keep_unused=True)

    def run(in_maps):
        concat_in = [
            np.concatenate([np.asarray(in_maps[c][nm]) for c in range(n_cores)],
                           axis=0)
            for nm in in_names
        ]
        concat_zero = [np.concatenate([z] * n_cores, axis=0) for z in zero_outs]
        outs = [np.asarray(o) for o in sharded(*concat_in, *concat_zero)]
        results = []
        for c in range(n_cores):
            r = {}
            for i, nm in enumerate(out_names):
                per = outs[i].shape[0] // n_cores
                r[nm] = outs[i][c * per:(c + 1) * per]
            results.append(r)
        return results

    return run


def _get_built(debug=False):
    key = "dbg" if debug else "main"
    if key not in _CACHE:
        nc = build_nc(debug=debug)
        run = _make_runner(nc, NCORES)
        _CACHE[key] = (nc, run)
    return _CACHE[key]


def kernel(**inputs):
    nc, run = _get_built()
    in_maps = _prep_inputs(inputs)
    results = run(in_maps)
    out3 = _assemble(results)
    return (np.asarray(inputs["x"], np.float32), out3)


# revision 32
# speedup vs baseline: 1.1200x; 1.0115x over previous
"""Trainium2 Bass kernel for nn_DecoderStack (2-layer decoder + FFN).

B=4 T=1024 D=1024 H=16 DK=DV=64 FF=4096, fp32 I/O.

Sharding (8 cores): core c -> batch b=c//2, head-group m=c%2 (8 of 16 heads).
Activations kept transposed on device: [d on partitions, t on free].
Matmul operands are bf16 (fp32 PSUM accumulation); stats/normalization in fp32.
Cross-core: per-pair AllReduce of the Wo1 partial; per-pair ReduceScatter of
(Wo2 partial + out1/2) giving each core its residual-included t-half for the
locally computed FFN.

kernel(**inputs) takes full unsharded inputs, returns (x, out3) like the ref.
"""
import contextlib

import numpy as np

import concourse.bass as bass  # noqa: F401
import concourse.tile as tile
from concourse import bacc, mybir

F32 = mybir.dt.float32
F32R = mybir.dt.float32r
BF16 = mybir.dt.bfloat16
AF = mybir.ActivationFunctionType
ALU = mybir.AluOpType

NCORES = 8
B, T, D, H, DK, DV, FF = 4, 1024, 1024, 16, 64, 64, 4096
NCH = D // 128          # 8 d-chunks of 128
NPAIR = 4               # head-pairs per core (8 heads)
THALF = T // 2
NFT = FF // 128         # 32 f-tiles
ISQ = float(1.0 / np.sqrt(np.float32(DK)))
INV_D = float(1.0 / D)
INV_D1 = float(1.0 / (D - 1))
D_OVER_D1 = float(D / (D - 1))

REPLICA_GROUPS = [[0, 1], [2, 3], [4, 5], [6, 7]]


# ================================================================ builder ===
def build_nc(debug=False):
    nc = bacc.Bacc("TRN2", target_bir_lowering=False, debug=False,
                   num_devices=NCORES)

    io = {}

    def din(name, shape, dt):
        io[name] = nc.dram_tensor(name, shape, dt, kind="ExternalInput")

    din("yT", [NCH, 128, T], BF16)
    din("yT32", [NCH, 128, T], F32)
    din("xT", [NCH, 128, T], BF16)
    din("wq1", [NCH, 128, 512], BF16)
    din("wk1", [NCH, 128, 512], BF16)
    din("wv1", [NCH, 128, 512], BF16)
    din("wo1", [2 * NPAIR, 128, D], BF16)
    din("wq2", [NCH, 128, 512], BF16)
    din("wk2", [NCH, 128, 512], BF16)
    din("wv2", [NCH, 128, 512], BF16)
    din("wo2", [NPAIR, 128, D], BF16)
    din("win", [NFT, NCH, 128, 128], BF16)
    din("wout", [NCH, NFT, 128, 128], BF16)
    din("bin", [128, NFT], F32)
    din("bout", [128, NCH], F32)
    din("mask", [128, 128], BF16)

    out_d = nc.dram_tensor("out3T", [NCH, 128, THALF], F32, kind="ExternalOutput")
    dbg = {}
    if debug:
        for name, shape, dt in (
                ("d_out1T", [128, NCH, T], BF16),
                ("d_qt1", [128, NPAIR, T], BF16),
                ("d_kt1", [128, NPAIR, T], BF16),
                ("d_vv1", [128, NCH, 512], BF16),
                ("d_aot1", [128, NPAIR, T], BF16),
                ("d_out2T", [128, NCH, THALF], BF16),
                ("d_z2", [128, NCH, THALF], BF16)):
            dbg[name] = nc.dram_tensor(name, shape, dt, kind="ExternalOutput")

    with tile.TileContext(nc) as tc:
        _emit(nc, tc, io, out_d, dbg)
    nc.compile()
    return nc


def _dump(nc, dbg, name, t_sb):
    if name in dbg:
        # stage through fp32 copy in DRAM-compatible layout
        nc.sync.dma_start(dbg[name].ap(), t_sb)


def _emit(nc, tc, io, out_d, dbg):
    ctx = contextlib.ExitStack()
    with ctx:
        # ---------------- outer pools (live whole kernel) ----------------
        const = ctx.enter_context(tc.tile_pool(name="const", bufs=1))
        stat = ctx.enter_context(tc.tile_pool(name="stat", bufs=1))
        scr = ctx.enter_context(tc.tile_pool(name="scr", bufs=1))
        epool = ctx.enter_context(tc.tile_pool(name="epool", bufs=1))
        hz = ctx.enter_context(tc.tile_pool(name="hz", bufs=1))
        dram = ctx.enter_context(tc.tile_pool(name="dram", bufs=1, space="DRAM"))
        psp = ctx.enter_context(tc.tile_pool(name="psp", bufs=1, space="PSUM"))

        def ps_tile(shape=(128, T)):
            return psp.tile(list(shape), F32, tag="ps", bufs=4, name="ps")

        # ---------------- constants ----------------
        ones_col = const.tile([128, 1], BF16)
        nc.vector.memset(ones_col[:], 1.0)
        ones_row = const.tile([1, 128], F32)
        nc.vector.memset(ones_row[:], 1.0)
        one1 = const.tile([1, 1], F32)
        nc.vector.memset(one1[:], 1.0)
        mask_sb = const.tile([128, 128], BF16)
        nc.sync.dma_start(mask_sb[:], io["mask"].ap())
        bin_sb = const.tile([128, NFT], F32)
        nc.sync.dma_start(bin_sb[:], io["bin"].ap())
        bout_sb = const.tile([128, NCH], F32)
        nc.sync.dma_start(bout_sb[:], io["bout"].ap())

        # =========== transposed-space layer norm (over d = partitions) =====
        def norm_into(z_sb, tw, out_sb, chunk_writer=None, off=0,
                      apply_src=None):
            """out_sb[:, c, :tw] = (z - mean)/std per t-column. z_sb bf16
            [128, NCH, tw]; out_sb may alias z_sb (chunk-wise in-place).
            If chunk_writer is given, per-chunk f32 results are handed to it
            instead of writing out_sb."""
            s_ps = ps_tile((1, T))
            for c in range(NCH):
                for nh in range(0, tw, 512):
                    w = min(512, tw - nh)
                    nc.tensor.matmul(
                        s_ps[:, nh:nh + w], ones_col[:],
                        z_sb[:, c, off + nh:off + nh + w],
                        start=(c == 0), stop=(c == NCH - 1))
            ss_ps = ps_tile((1, T))
            for c in range(NCH):
                zsq = scr.tile([128, T], BF16, tag="zsq", bufs=2, name="zsq")
                nc.vector.tensor_mul(out=zsq[:, :tw],
                                     in0=z_sb[:, c, off:off + tw],
                                     in1=z_sb[:, c, off:off + tw])
                for nh in range(0, tw, 512):
                    w = min(512, tw - nh)
                    nc.tensor.matmul(
                        ss_ps[:, nh:nh + w], ones_col[:], zsq[:, nh:nh + w],
                        start=(c == 0), stop=(c == NCH - 1))
            mean = stat.tile([1, T], F32, tag="mean", bufs=1, name="mean")
            nc.vector.tensor_scalar(out=mean[:, :tw], in0=s_ps[:, :tw],
                                    scalar1=INV_D, scalar2=None, op0=ALU.mult)
            m2 = stat.tile([1, T], F32, tag="m2", bufs=1, name="m2")
            nc.vector.tensor_mul(out=m2[:, :tw], in0=mean[:, :tw],
                                 in1=mean[:, :tw])
            var = stat.tile([1, T], F32, tag="var", bufs=1, name="var")
            nc.vector.tensor_scalar(out=var[:, :tw], in0=ss_ps[:, :tw],
                                    scalar1=INV_D1, scalar2=None, op0=ALU.mult)
            nc.vector.tensor_scalar(out=m2[:, :tw], in0=m2[:, :tw],
                                    scalar1=D_OVER_D1, scalar2=None,
                                    op0=ALU.mult)
            nc.vector.tensor_tensor(var[:, :tw], var[:, :tw], m2[:, :tw],
                                    ALU.subtract)
            nc.scalar.activation(out=var[:, :tw], in_=var[:, :tw], func=AF.Sqrt)
            rstd = stat.tile([1, T], F32, tag="rstd", bufs=1, name="rstd")
            nc.vector.reciprocal(out=rstd[:, :tw], in_=var[:, :tw])
            nmr = stat.tile([1, T], F32, tag="nmr", bufs=1, name="nmr")
            nc.vector.tensor_mul(out=nmr[:, :tw], in0=mean[:, :tw],
                                 in1=rstd[:, :tw])
            nc.vector.tensor_scalar(out=nmr[:, :tw], in0=nmr[:, :tw],
                                    scalar1=-1.0, scalar2=None, op0=ALU.mult)
            rstd_ps = ps_tile()
            nmr_ps = ps_tile()
            for nh in range(0, tw, 512):
                w = min(512, tw - nh)
                nc.tensor.matmul(rstd_ps[:, nh:nh + w], ones_row[:],
                                 rstd[:, nh:nh + w], start=True, stop=True)
                nc.tensor.matmul(nmr_ps[:, nh:nh + w], ones_row[:],
                                 nmr[:, nh:nh + w], start=True, stop=True)
            rstd_bc = scr.tile([128, T], F32, tag="bc", bufs=2, name="rstd_bc")
            nc.vector.tensor_copy(out=rstd_bc[:, :tw], in_=rstd_ps[:, :tw])
            nmr_bc = scr.tile([128, T], F32, tag="bc", bufs=2, name="nmr_bc")
            nc.vector.tensor_copy(out=nmr_bc[:, :tw], in_=nmr_ps[:, :tw])
            a_src = z_sb if apply_src is None else apply_src
            for c in range(NCH):
                tmp = scr.tile([128, T], F32, tag="s4", bufs=4, name="ntmp")
                nc.vector.tensor_mul(out=tmp[:, :tw],
                                     in0=a_src[:, c, off:off + tw],
                                     in1=rstd_bc[:, :tw])
                if chunk_writer is None:
                    nc.vector.tensor_tensor(out_sb[:, c, off:off + tw], tmp[:, :tw],
                                            nmr_bc[:, :tw], ALU.add)
                else:
                    oc = scr.tile([128, T], F32, tag="s4", bufs=4, name="oc")
                    nc.vector.tensor_tensor(oc[:, :tw], tmp[:, :tw],
                                            nmr_bc[:, :tw], ALU.add)
                    chunk_writer(c, oc[:, :tw])

        # ================= attention inner block (scores/exp/AV) ===========
        # proj_spec: list of (dst, w_t, src, fold) projected per-pair right
        # before that pair's scores — keeps PE dense while ACT runs exps.
        def attn_inner(qt_sb, kt_sb, vv_sb, aot_sb, proj_spec=()):
            for p in range(NPAIR):
                for dst, w_t, src, fld in proj_spec:
                    pp = ps_tile()
                    for c in range(NCH):
                        lhsT = w_t[:, c, 128 * p:128 * (p + 1)]
                        for nh in range(2):
                            nc.tensor.matmul(
                                pp[:, 512 * nh:512 * (nh + 1)], lhsT,
                                src[:, c, 512 * nh:512 * (nh + 1)],
                                start=(c == 0), stop=(c == NCH - 1))
                    if fld is None:
                        nc.vector.tensor_copy(out=dst[:, p, :], in_=pp[:])
                    else:
                        nc.vector.tensor_scalar(
                            out=dst[:, p, :], in0=pp[:], scalar1=fld[p][:],
                            scalar2=None, op0=ALU.mult)
                av_ps = ps_tile()

                def emit_av(st, e_pair, zp):
                    rp = stat.tile([128, 2], F32, tag="rp", bufs=4, name="rp")
                    nc.vector.reciprocal(out=rp[:], in_=zp[:])
                    vv_sc = scr.tile([128, 2, 64], BF16, tag="vvsc", bufs=3,
                                     name="vv_sc")
                    nc.vector.tensor_tensor(
                        vv_sc[:],
                        vv_sb[:, st, 128 * p:128 * (p + 1)].rearrange(
                            "s (h v) -> s h v", h=2),
                        rp[:, :, None].to_broadcast([128, 2, 64]),
                        ALU.mult)
                    for h in range(2):
                        for nh in range(2):
                            nc.tensor.matmul(
                                av_ps[64 * h:64 * (h + 1),
                                      512 * nh:512 * (nh + 1)],
                                vv_sc[:, h, :],
                                e_pair[h][:, 512 * nh:512 * (nh + 1)],
                                start=(st == 0), stop=(st == NCH - 1),
                                tile_position=(0, 64 * h))

                prev = None  # one-step software pipeline: scores(st+1) issue
                for st in range(NCH):  # before AV(st)'s exp-gated wait
                    zp = stat.tile([128, 2], F32, tag="zp", bufs=4, name="zp")
                    e_pair = []
                    for h in range(2):
                        sc_ps = ps_tile()
                        k0 = 64 * h
                        lhsT = kt_sb[k0:k0 + 64, p, 128 * st:128 * (st + 1)]
                        for nh in range(2):
                            nc.tensor.matmul(
                                sc_ps[:, 512 * nh:512 * (nh + 1)], lhsT,
                                qt_sb[k0:k0 + 64, p, 512 * nh:512 * (nh + 1)],
                                start=True, stop=True, tile_position=(k0, 0))
                        e_st = epool.tile([128, T], BF16, tag="E", bufs=4,
                                          name="e_st")
                        nc.scalar.activation(
                            out=e_st[:], in_=sc_ps[:], func=AF.Exp, scale=ISQ,
                            accum_out=zp[:, h:h + 1])
                        e_pair.append(e_st)
                    if prev is not None:
                        emit_av(*prev)
                    prev = (st, e_pair, zp)
                emit_av(*prev)
                nc.vector.tensor_copy(out=aot_sb[:, p, :], in_=av_ps[:])

        # ================= projection helpers ==============================
        def proj_qk(dst, w_t, src, fold):
            """dst[:, p, :] = (W_pair^T @ src) [optionally * fold[p] rows]."""
            for p in range(NPAIR):
                pp = ps_tile()
                for c in range(NCH):
                    lhsT = w_t[:, c, 128 * p:128 * (p + 1)]
                    for nh in range(2):
                        nc.tensor.matmul(
                            pp[:, 512 * nh:512 * (nh + 1)], lhsT,
                            src[:, c, 512 * nh:512 * (nh + 1)],
                            start=(c == 0), stop=(c == NCH - 1))
                if fold is None:
                    nc.vector.tensor_copy(out=dst[:, p, :], in_=pp[:])
                else:
                    nc.vector.tensor_scalar(
                        out=dst[:, p, :], in0=pp[:], scalar1=fold[p][:],
                        scalar2=None, op0=ALU.mult)

        def proj_v(dst, w_t, src):
            for st in range(NCH):
                vp = ps_tile((128, 512))
                for c in range(NCH):
                    nc.tensor.matmul(
                        vp[:], src[:, c, 128 * st:128 * (st + 1)],
                        w_t[:, c, :], start=(c == 0), stop=(c == NCH - 1))
                nc.vector.tensor_copy(out=dst[:, st, :], in_=vp[:])

        def wo_partial(wo_sb, aot_sb, emit_e_tile):
            for e in range(NCH):
                wo_ps = ps_tile()
                for p in range(NPAIR):
                    lhsT = wo_sb[:, p, 128 * e:128 * (e + 1)]
                    for nh in range(2):
                        nc.tensor.matmul(
                            wo_ps[:, 512 * nh:512 * (nh + 1)], lhsT,
                            aot_sb[:, p, 512 * nh:512 * (nh + 1)],
                            start=(p == 0), stop=(p == NPAIR - 1))
                emit_e_tile(e, wo_ps)

        def load_w(pool, name, tag):
            t = pool.tile([128, NCH, 512], BF16, tag=tag, name=name + "_sb")
            nc.sync.dma_start(t[:], io[name].ap().rearrange("c p k -> p c k"))
            return t

        # ============================ start =================================
        with tc.tile_pool(name="actA", bufs=1) as actA:
            y_sb = actA.tile([128, NCH, T], BF16, tag="y", name="y_sb")

            with tc.tile_pool(name="gio", bufs=1) as gio:
                qt = gio.tile([128, NPAIR, T], BF16, tag="qt", name="qt")
                kt = gio.tile([128, NPAIR, T], BF16, tag="kt", name="kt")
                vv = gio.tile([128, NCH, 512], BF16, tag="vv", name="vv")
                aot = gio.tile([128, NPAIR, T], BF16, tag="aot", name="aot")

                ag_in = dram.tile([NPAIR, 128, T], BF16, tag="ag_in",
                                  name="ag_in")
                ag_out = dram.tile([2, NPAIR, 128, T], BF16, tag="ag_out",
                                   name="ag_out")
                TQ = THALF // 2
                rs_inA = dram.tile([2, NCH, 128, TQ], BF16, tag="rs_inA",
                                   name="rs_inA")
                rs_inB = dram.tile([2, NCH, 128, TQ], BF16, tag="rs_inB",
                                   name="rs_inB")
                rs_outA = dram.tile([NCH, 128, TQ], BF16, tag="rs_outA",
                                    name="rs_outA")
                rs_outB = dram.tile([NCH, 128, TQ], BF16, tag="rs_outB",
                                    name="rs_outB")

                # ---------------- Layer 1 ----------------
                with tc.tile_pool(name="w1", bufs=1) as w1:
                    # masked weight softmax for Wq1/Wk1 (no max-subtraction)
                    ewq = w1.tile([128, NCH, 512], BF16, tag="ewq", name="ewq")
                    ewk = w1.tile([128, NCH, 512], BF16, tag="ewk", name="ewk")
                    for nm, ew in (("wq1", ewq), ("wk1", ewk)):
                        raw = w1.tile([128, NCH, 512], BF16, tag="wraw",
                                      bufs=1, name="wraw")
                        nc.sync.dma_start(
                            raw[:], io[nm].ap().rearrange("c p k -> p c k"))
                        nc.scalar.activation(out=ew[:], in_=raw[:], func=AF.Exp)
                        nc.vector.tensor_tensor(
                            ew[:, 0, :].rearrange("p (q k) -> p q k", q=NPAIR),
                            ew[:, 0, :].rearrange("p (q k) -> p q k", q=NPAIR),
                            mask_sb[:, None, :].to_broadcast([128, NPAIR, 128]),
                            ALU.mult)
                    nc.sync.dma_start(
                        y_sb[:], io["yT"].ap().rearrange("c p t -> p c t"))
                    # column sums over d -> ck = 1/(Sq*Sk), transposed per pair
                    sq_ps = ps_tile((1, 512))
                    for c in range(NCH):
                        nc.tensor.matmul(sq_ps[:], ones_col[:], ewq[:, c, :],
                                         start=(c == 0), stop=(c == NCH - 1))
                    sk_ps = ps_tile((1, 512))
                    for c in range(NCH):
                        nc.tensor.matmul(sk_ps[:], ones_col[:], ewk[:, c, :],
                                         start=(c == 0), stop=(c == NCH - 1))
                    rq = stat.tile([1, 512], F32, tag="rq", name="rq")
                    rk = stat.tile([1, 512], F32, tag="rk", name="rk")
                    nc.vector.reciprocal(out=rq[:], in_=sq_ps[:])
                    nc.vector.reciprocal(out=rk[:], in_=sk_ps[:])
                    ckk = stat.tile([1, 512], F32, tag="ck", name="ckk")
                    nc.vector.tensor_mul(out=ckk[:], in0=rq[:], in1=rk[:])
                    ckT = []
                    for p in range(NPAIR):
                        ct_ps = ps_tile((128, 1))
                        nc.tensor.matmul(ct_ps[:],
                                         ckk[:, 128 * p:128 * (p + 1)],
                                         one1[:], start=True, stop=True)
                        ct = stat.tile([128, 1], F32, tag=f"ckT{p}",
                                       name=f"ckT{p}")
                        nc.vector.tensor_copy(out=ct[:], in_=ct_ps[:])
                        ckT.append(ct)

                    wv1 = load_w(w1, "wv1", "wv")
                    proj_v(vv, wv1, y_sb)

                    proj_qk(qt, ewq, y_sb, None)
                    proj_qk(kt, ewk, y_sb, ckT)
                    attn_inner(qt, kt, vv, aot)
                    _dump(nc, dbg, "d_qt1", qt[:])
                    _dump(nc, dbg, "d_kt1", kt[:])
                    _dump(nc, dbg, "d_vv1", vv[:])
                    _dump(nc, dbg, "d_aot1", aot[:])
                    nc.sync.dma_start(ag_in.rearrange("q p t -> p q t"),
                                      aot[:])
                nc.gpsimd.collective_compute(
                    "AllGather", ALU.bypass, replica_groups=REPLICA_GROUPS,
                    ins=[ag_in.opt()], outs=[ag_out.opt()])

                # ---------------- Layer 2 (overlaps the AllReduce) ---------
                with tc.tile_pool(name="actB", bufs=1) as actB:
                    x_sb = actB.tile([128, NCH, T], BF16, tag="x", name="x_sb")
                    nc.sync.dma_start(
                        x_sb[:], io["xT"].ap().rearrange("c p t -> p c t"))
                    with tc.tile_pool(name="w2", bufs=1) as w2:
                        wq2 = load_w(w2, "wq2", "wq2")
                        wk2 = load_w(w2, "wk2", "wk2")
                        wv2 = load_w(w2, "wv2", "wv2")
                        wo2 = w2.tile([128, NPAIR, D], BF16, tag="wo2",
                                      name="wo2")
                        nc.sync.dma_start(
                            wo2[:], io["wo2"].ap().rearrange("q p e -> p q e"))
                        # K/V projections only need x -> run during the AR
                        proj_qk(kt, wk2, x_sb, None)
                        proj_v(vv, wv2, x_sb)

                        # gather both cores' AOT, full Wo1 locally;
                        # z1 = Wo1(aot_full) + y(fp32), in place into y_sb
                        wo1f = w2.tile([128, 2, NPAIR, D], BF16, tag="wo1f",
                                       name="wo1f")
                        nc.sync.dma_start(
                            wo1f[:], io["wo1"].ap()
                            .rearrange("(r q) p e -> p r q e", r=2))
                        aot_full = w2.tile([128, 2, NPAIR, T], BF16,
                                           tag="aotf", name="aot_full")
                        nc.sync.dma_start(
                            aot_full[:],
                            ag_out.rearrange("r q p t -> p r q t"))
                        for e in range(NCH):
                            wo_ps = ps_tile()
                            for r in range(2):
                                for p in range(NPAIR):
                                    lhsT = wo1f[:, r, p, 128 * e:128 * (e + 1)]
                                    for nh in range(2):
                                        nc.tensor.matmul(
                                            wo_ps[:, 512 * nh:512 * (nh + 1)],
                                            lhsT,
                                            aot_full[:, r, p,
                                                     512 * nh:512 * (nh + 1)],
                                            start=(r == 0 and p == 0),
                                            stop=(r == 1 and p == NPAIR - 1))
                            yf = scr.tile([128, T], F32, tag="s4", bufs=4,
                                          name="yf")
                            nc.sync.dma_start(yf[:], io["yT32"].ap()[e])
                            nc.vector.tensor_tensor(y_sb[:, e, :], wo_ps[:],
                                                    yf[:], ALU.add)
                        out1 = y_sb  # alias: z1 normalized in place
                        norm_into(y_sb, T, out1)
                        _dump(nc, dbg, "d_out1T", out1[:])

                        proj_qk(qt, wq2, out1, None)
                        attn_inner(qt, kt, vv, aot)

                        def emit_rs(e, wo_ps):
                            half = scr.tile([128, T], BF16, tag="sb4", bufs=2,
                                            name="half")
                            nc.vector.tensor_scalar(
                                out=half[:], in0=out1[:, e, :], scalar1=0.5,
                                scalar2=None, op0=ALU.mult)
                            res = scr.tile([128, T], BF16, tag="sb4", bufs=2,
                                           name="res")
                            nc.vector.tensor_tensor(res[:], wo_ps[:], half[:],
                                                    ALU.add)
                            nc.sync.dma_start(rs_inA[0, e], res[:, 0:TQ])
                            nc.sync.dma_start(rs_inA[1, e],
                                              res[:, THALF:THALF + TQ])
                            nc.sync.dma_start(rs_inB[0, e], res[:, TQ:THALF])
                            nc.sync.dma_start(rs_inB[1, e], res[:, THALF + TQ:])

                        wo_partial(wo2, aot, emit_rs)
                nc.gpsimd.collective_compute(
                    "ReduceScatter", ALU.add, replica_groups=REPLICA_GROUPS,
                    ins=[rs_inA.opt()], outs=[rs_outA.opt()])
                nc.gpsimd.collective_compute(
                    "ReduceScatter", ALU.add, replica_groups=REPLICA_GROUPS,
                    ins=[rs_inB.opt()], outs=[rs_outB.opt()])

            # ---------------- FFN on local t-half ----------------
            with tc.tile_pool(name="fh", bufs=1) as fh:
                h_sb = fh.tile([128, NFT, THALF], BF16, tag="h", name="h_sb")
                NWIN = 4
                pre_win = {}
                for ft in range(NWIN):
                    wt = fh.tile([128, NCH, 128], BF16, tag="win",
                                 bufs=NWIN, name="win_t")
                    nc.sync.dma_start(
                        wt[:], io["win"].ap()[ft].rearrange("c p f -> p c f"))
                    pre_win[ft] = wt
                pre_wout = fh.tile([128, NFT, 128], BF16, tag="wout", bufs=2,
                                   name="wout_t")
                nc.sync.dma_start(pre_wout[:],
                                  io["wout"].ap()[0].rearrange("f p m -> p f m"))

                TQ = THALF // 2
                z2 = hz.tile([128, NCH, THALF], BF16, tag="hz", bufs=1, name="z2")
                nc.sync.dma_start(z2[:, :, 0:TQ],
                                  rs_outA.rearrange("c p t -> p c t"))
                out2 = z2
                norm_into(z2, TQ, out2, off=0)
                nc.sync.dma_start(z2[:, :, TQ:THALF],
                                  rs_outB.rearrange("c p t -> p c t"))
                win_tiles = {}
                for ft in range(NFT):
                    if ft in pre_win:
                        win_t = pre_win.pop(ft)
                    else:
                        win_t = fh.tile([128, NCH, 128], BF16, tag="win",
                                        bufs=NWIN, name="win_t")
                        nc.sync.dma_start(
                            win_t[:],
                            io["win"].ap()[ft].rearrange("c p f -> p c f"))
                    win_tiles[ft] = win_t
                    hp = ps_tile((128, THALF))
                    for c in range(NCH):
                        nc.tensor.matmul(hp[:, 0:TQ], win_t[:, c, :],
                                         out2[:, c, 0:TQ],
                                         start=(c == 0), stop=(c == NCH - 1))
                    nc.scalar.activation(out=h_sb[:, ft, 0:TQ], in_=hp[:, 0:TQ],
                                         func=AF.Relu,
                                         bias=bin_sb[:, ft:ft + 1], scale=1.0)
                    if ft == 0:
                        norm_into(z2, TQ, out2, off=TQ)
                    hp2 = ps_tile((128, THALF))
                    for c in range(NCH):
                        nc.tensor.matmul(hp2[:, 0:TQ], win_t[:, c, :],
                                         out2[:, c, TQ:THALF],
                                         start=(c == 0), stop=(c == NCH - 1))
                    nc.scalar.activation(out=h_sb[:, ft, TQ:THALF],
                                         in_=hp2[:, 0:TQ], func=AF.Relu,
                                         bias=bin_sb[:, ft:ft + 1], scale=1.0)
                    del win_tiles[ft]

                z3 = out2  # in-place: out2[:, e] is last read by this add
                z3f = fh.tile([128, NCH, THALF], F32, tag="z3f", name="z3f")
                for e in range(NCH):
                    if e == 0:
                        wout_t = pre_wout
                    else:
                        wout_t = fh.tile([128, NFT, 128], BF16,
                                         tag="wout", bufs=2, name="wout_t")
                        nc.sync.dma_start(
                            wout_t[:],
                            io["wout"].ap()[e].rearrange("f p m -> p f m"))
                    fp = ps_tile((128, THALF))
                    for fc in range(NFT):
                        nc.tensor.matmul(
                            fp[:], wout_t[:, fc, :], h_sb[:, fc, :],
                            start=(fc == 0), stop=(fc == NFT - 1))
                    t1 = scr.tile([128, T], F32, tag="s4", bufs=4, name="fftmp")
                    nc.vector.tensor_scalar(out=t1[:, :THALF], in0=fp[:],
                                            scalar1=bout_sb[:, e:e + 1],
                                            scalar2=None, op0=ALU.add)
                    nc.vector.tensor_tensor(z3f[:, e, :], t1[:, :THALF],
                                            out2[:, e, :], ALU.add)
                    nc.vector.tensor_copy(out=z3[:, e, :], in_=z3f[:, e, :])
                norm_into(z3, THALF, None,
                          chunk_writer=lambda c, oc: nc.sync.dma_start(
                              out_d.ap()[c], oc), apply_src=z3f)


# ============================================================== host side ===
def _to_bf16(a):
    import ml_dtypes
    return np.asarray(a, np.float32).astype(ml_dtypes.bfloat16)


def _prep_inputs(inputs):
    """Per-core in_maps (host does transposes/tiling/dtype casts only)."""
    x = np.asarray(inputs["x"], np.float32)
    y = np.asarray(inputs["y"], np.float32)
    mask = _to_bf16(np.tile(np.tril(np.ones((128, DK), np.float32)), (1, 2)))
    win_t = _to_bf16(np.asarray(inputs["w_in"], np.float32).T
                     .reshape(NCH, 128, NFT, 128).transpose(2, 0, 1, 3))
    wout_t = _to_bf16(np.asarray(inputs["w_out"], np.float32).T
                      .reshape(NFT, 128, NCH, 128).transpose(2, 0, 1, 3))
    bin2 = np.ascontiguousarray(
        np.asarray(inputs["b_in"], np.float32).reshape(NFT, 128).T)
    bout2 = np.ascontiguousarray(
        np.asarray(inputs["b_out"], np.float32).reshape(NCH, 128).T)

    def packw(w, hs):
        return _to_bf16(np.asarray(w, np.float32)[hs].transpose(1, 0, 2)
                        .reshape(D, 512).reshape(NCH, 128, 512))

    shared = {"win": win_t, "wout": wout_t, "bin": bin2, "bout": bout2,
              "mask": mask}
    in_maps = []
    for c in range(NCORES):
        b, m = c // 2, c % 2
        hs = slice(8 * m, 8 * (m + 1))
        im = dict(shared)
        im["yT"] = _to_bf16(y[b].T.reshape(NCH, 128, T))
        im["yT32"] = np.ascontiguousarray(y[b].T.reshape(NCH, 128, T))
        im["xT"] = _to_bf16(x[b].T.reshape(NCH, 128, T))
        im["wq1"] = packw(inputs["Wq1"], hs)
        im["wk1"] = packw(inputs["Wk1"], hs)
        im["wv1"] = packw(inputs["Wv1"], hs)
        im["wq2"] = packw(inputs["Wq2"], hs)
        im["wk2"] = packw(inputs["Wk2"], hs)
        im["wv2"] = packw(inputs["Wv2"], hs)
        im["wo1"] = _to_bf16(np.asarray(inputs["Wo1"], np.float32)
                             .reshape(2 * NPAIR, 128, D))
        im["wo2"] = _to_bf16(np.asarray(inputs["Wo2"], np.float32)
                             [512 * m:512 * (m + 1)].reshape(NPAIR, 128, D))
        in_maps.append(im)
    return in_maps


def _assemble(results):
    out3 = np.empty((B, T, D), np.float32)
    for b in range(B):
        halves = [results[2 * b + m]["out3T"].reshape(D, THALF)
                  for m in range(2)]
        out3[b] = np.concatenate(halves, axis=1).T
    return out3


# ================================================================ runner ===
_CACHE = {}


def _make_runner(nc, n_cores):
    import jax
    from jax.sharding import Mesh, PartitionSpec
    from jax.experimental.shard_map import shard_map
    from concourse.bass2jax import (_bass_exec_p, install_neuronx_cc_hook,
                                    partition_id_tensor)

    install_neuronx_cc_hook()
    partition_name = nc.partition_id_tensor.name if nc.partition_id_tensor else None
    in_names, out_names, out_avals, zero_outs = [], [], [], []
    for alloc in nc.m.functions[0].allocations:
        if not isinstance(alloc, mybir.MemoryLocationSet):
            continue
        name = alloc.memorylocations[0].name
        if alloc.kind == "ExternalInput":
            if name != partition_name:
                in_names.append(name)
        elif alloc.kind == "ExternalOutput":
            shape = tuple(alloc.tensor_shape)
            dtype = mybir.dt.np(alloc.dtype)
            out_names.append(name)
            out_avals.append(jax.core.ShapedArray(shape, dtype))
            zero_outs.append(np.zeros(shape, dtype))
    n_params = len(in_names)
    n_outs = len(out_avals)
    all_in = in_names + out_names + ([partition_name] if partition_name else [])

    def _body(*args):
        operands = list(args)
        if partition_name is not None:
            operands.append(partition_id_tensor())
        return tuple(_bass_exec_p.bind(
            *operands, out_avals=tuple(out_avals), in_names=tuple(all_in),
            out_names=tuple(out_names), lowering_input_output_aliases=(),
            sim_require_finite=True, sim_require_nnan=True, nc=nc))

    devices = jax.devices()[:n_cores]
    mesh = Mesh(np.asarray(devices), ("core",))
    sharded = jax.jit(
        shard_map(_body, mesh=mesh,
                  in_specs=(PartitionSpec("core"),) * (n_params + n_outs),
                  out_specs=(PartitionSpec("core"),) * n_outs,
                  check_rep=False),
        keep_unused=True)

    def run(in_maps):
        concat_in = [
            np.concatenate([np.asarray(in_maps[c][nm]) for c in range(n_cores)],
                           axis=0)
            for nm in in_names
        ]
        concat_zero = [np.concatenate([z] * n_cores, axis=0) for z in zero_outs]
        outs = [np.asarray(o) for o in sharded(*concat_in, *concat_zero)]
        results = []
        for c in range(n_cores):
            r = {}
            for i, nm in enumerate(out_names):
                per = outs[i].shape[0] // n_cores
                r[nm] = outs[i][c * per:(c + 1) * per]
            results.append(r)
        return results

    return run


def _get_built(debug=False):
    key = "dbg" if debug else "main"
    if key not in _CACHE:
        nc = build_nc(debug=debug)
        run = _make_runner(nc, NCORES)
        _CACHE[key] = (nc, run)
    return _CACHE[key]


def kernel(**inputs):
    nc, run = _get_built()
    in_maps = _prep_inputs(inputs)
    results = run(in_maps)
    out3 = _assemble(results)
    return (np.asarray(inputs["x"], np.float32), out3)


# revision 33
# speedup vs baseline: 1.1631x; 1.0385x over previous
"""Trainium2 Bass kernel for nn_DecoderStack (2-layer decoder + FFN).

B=4 T=1024 D=1024 H=16 DK=DV=64 FF=4096, fp32 I/O.

Sharding (8 cores): core c -> batch b=c//2, head-group m=c%2 (8 of 16 heads).
Activations kept transposed on device: [d on partitions, t on free].
Matmul operands are bf16 (fp32 PSUM accumulation); stats/normalization in fp32.
Cross-core: per-pair AllReduce of the Wo1 partial; per-pair ReduceScatter of
(Wo2 partial + out1/2) giving each core its residual-included t-half for the
locally computed FFN.

kernel(**inputs) takes full unsharded inputs, returns (x, out3) like the ref.
"""
import contextlib

import numpy as np

import concourse.bass as bass  # noqa: F401
import concourse.tile as tile
from concourse import bacc, mybir

F32 = mybir.dt.float32
F32R = mybir.dt.float32r
BF16 = mybir.dt.bfloat16
AF = mybir.ActivationFunctionType
ALU = mybir.AluOpType

NCORES = 8
B, T, D, H, DK, DV, FF = 4, 1024, 1024, 16, 64, 64, 4096
NCH = D // 128          # 8 d-chunks of 128
NPAIR = 4               # head-pairs per core (8 heads)
THALF = T // 2
NFT = FF // 128         # 32 f-tiles
ISQ = float(1.0 / np.sqrt(np.float32(DK)))
INV_D = float(1.0 / D)
INV_D1 = float(1.0 / (D - 1))
D_OVER_D1 = float(D / (D - 1))

REPLICA_GROUPS = [[0, 1], [2, 3], [4, 5], [6, 7]]


# ================================================================ builder ===
def build_nc(debug=False):
    nc = bacc.Bacc("TRN2", target_bir_lowering=False, debug=False,
                   num_devices=NCORES)

    io = {}

    def din(name, shape, dt):
        io[name] = nc.dram_tensor(name, shape, dt, kind="ExternalInput")

    din("yT", [NCH, 128, T], BF16)
    din("yT32", [NCH, 128, T], F32)
    din("xT", [NCH, 128, T], BF16)
    din("wq1", [NCH, 128, 512], BF16)
    din("wk1", [NCH, 128, 512], BF16)
    din("wv1", [NCH, 128, 512], BF16)
    din("wo1", [2 * NPAIR, 128, D], BF16)
    din("wq2", [NCH, 128, 512], BF16)
    din("wk2", [NCH, 128, 512], BF16)
    din("wv2", [NCH, 128, 512], BF16)
    din("wo2", [NPAIR, 128, D], BF16)
    din("win", [NFT, NCH, 128, 128], BF16)
    din("wout", [NCH, NFT, 128, 128], BF16)
    din("bin", [128, NFT], F32)
    din("bout", [128, NCH], F32)
    din("mask", [128, 128], BF16)

    out_d = nc.dram_tensor("out3T", [NCH, 128, THALF], F32, kind="ExternalOutput")
    dbg = {}
    if debug:
        for name, shape, dt in (
                ("d_out1T", [128, NCH, T], BF16),
                ("d_qt1", [128, NPAIR, T], BF16),
                ("d_kt1", [128, NPAIR, T], BF16),
                ("d_vv1", [128, NCH, 512], BF16),
                ("d_aot1", [128, NPAIR, T], BF16),
                ("d_out2T", [128, NCH, THALF], BF16),
                ("d_z2", [128, NCH, THALF], BF16)):
            dbg[name] = nc.dram_tensor(name, shape, dt, kind="ExternalOutput")

    with tile.TileContext(nc) as tc:
        _emit(nc, tc, io, out_d, dbg)
    nc.compile()
    return nc


def _dump(nc, dbg, name, t_sb):
    if name in dbg:
        # stage through fp32 copy in DRAM-compatible layout
        nc.sync.dma_start(dbg[name].ap(), t_sb)


def _emit(nc, tc, io, out_d, dbg):
    ctx = contextlib.ExitStack()
    with ctx:
        # ---------------- outer pools (live whole kernel) ----------------
        const = ctx.enter_context(tc.tile_pool(name="const", bufs=1))
        stat = ctx.enter_context(tc.tile_pool(name="stat", bufs=1))
        scr = ctx.enter_context(tc.tile_pool(name="scr", bufs=1))
        epool = ctx.enter_context(tc.tile_pool(name="epool", bufs=1))
        hz = ctx.enter_context(tc.tile_pool(name="hz", bufs=1))
        dram = ctx.enter_context(tc.tile_pool(name="dram", bufs=1, space="DRAM"))
        psp = ctx.enter_context(tc.tile_pool(name="psp", bufs=1, space="PSUM"))

        def ps_tile(shape=(128, T)):
            return psp.tile(list(shape), F32, tag="ps", bufs=4, name="ps")

        # ---------------- constants ----------------
        ones_col = const.tile([128, 1], BF16)
        nc.vector.memset(ones_col[:], 1.0)
        ones_row = const.tile([1, 128], F32)
        nc.vector.memset(ones_row[:], 1.0)
        one1 = const.tile([1, 1], F32)
        nc.vector.memset(one1[:], 1.0)
        mask_sb = const.tile([128, 128], BF16)
        nc.sync.dma_start(mask_sb[:], io["mask"].ap())
        bin_sb = const.tile([128, NFT], F32)
        nc.sync.dma_start(bin_sb[:], io["bin"].ap())
        bout_sb = const.tile([128, NCH], F32)
        nc.sync.dma_start(bout_sb[:], io["bout"].ap())

        # =========== transposed-space layer norm (over d = partitions) =====
        def norm_into(z_sb, tw, out_sb, chunk_writer=None, off=0,
                      apply_src=None):
            """out_sb[:, c, :tw] = (z - mean)/std per t-column. z_sb bf16
            [128, NCH, tw]; out_sb may alias z_sb (chunk-wise in-place).
            If chunk_writer is given, per-chunk f32 results are handed to it
            instead of writing out_sb."""
            s_ps = ps_tile((1, T))
            for c in range(NCH):
                for nh in range(0, tw, 512):
                    w = min(512, tw - nh)
                    nc.tensor.matmul(
                        s_ps[:, nh:nh + w], ones_col[:],
                        z_sb[:, c, off + nh:off + nh + w],
                        start=(c == 0), stop=(c == NCH - 1))
            ss_ps = ps_tile((1, T))
            for c in range(NCH):
                zsq = scr.tile([128, T], BF16, tag="zsq", bufs=2, name="zsq")
                nc.vector.tensor_mul(out=zsq[:, :tw],
                                     in0=z_sb[:, c, off:off + tw],
                                     in1=z_sb[:, c, off:off + tw])
                for nh in range(0, tw, 512):
                    w = min(512, tw - nh)
                    nc.tensor.matmul(
                        ss_ps[:, nh:nh + w], ones_col[:], zsq[:, nh:nh + w],
                        start=(c == 0), stop=(c == NCH - 1))
            mean = stat.tile([1, T], F32, tag="mean", bufs=1, name="mean")
            nc.vector.tensor_scalar(out=mean[:, :tw], in0=s_ps[:, :tw],
                                    scalar1=INV_D, scalar2=None, op0=ALU.mult)
            m2 = stat.tile([1, T], F32, tag="m2", bufs=1, name="m2")
            nc.vector.tensor_mul(out=m2[:, :tw], in0=mean[:, :tw],
                                 in1=mean[:, :tw])
            var = stat.tile([1, T], F32, tag="var", bufs=1, name="var")
            nc.vector.tensor_scalar(out=var[:, :tw], in0=ss_ps[:, :tw],
                                    scalar1=INV_D1, scalar2=None, op0=ALU.mult)
            nc.vector.tensor_scalar(out=m2[:, :tw], in0=m2[:, :tw],
                                    scalar1=D_OVER_D1, scalar2=None,
                                    op0=ALU.mult)
            nc.vector.tensor_tensor(var[:, :tw], var[:, :tw], m2[:, :tw],
                                    ALU.subtract)
            nc.scalar.activation(out=var[:, :tw], in_=var[:, :tw], func=AF.Sqrt)
            rstd = stat.tile([1, T], F32, tag="rstd", bufs=1, name="rstd")
            nc.vector.reciprocal(out=rstd[:, :tw], in_=var[:, :tw])
            nmr = stat.tile([1, T], F32, tag="nmr", bufs=1, name="nmr")
            nc.vector.tensor_mul(out=nmr[:, :tw], in0=mean[:, :tw],
                                 in1=rstd[:, :tw])
            nc.vector.tensor_scalar(out=nmr[:, :tw], in0=nmr[:, :tw],
                                    scalar1=-1.0, scalar2=None, op0=ALU.mult)
            rstd_ps = ps_tile()
            nmr_ps = ps_tile()
            for nh in range(0, tw, 512):
                w = min(512, tw - nh)
                nc.tensor.matmul(rstd_ps[:, nh:nh + w], ones_row[:],
                                 rstd[:, nh:nh + w], start=True, stop=True)
                nc.tensor.matmul(nmr_ps[:, nh:nh + w], ones_row[:],
                                 nmr[:, nh:nh + w], start=True, stop=True)
            rstd_bc = scr.tile([128, T], F32, tag="bc", bufs=2, name="rstd_bc")
            nc.vector.tensor_copy(out=rstd_bc[:, :tw], in_=rstd_ps[:, :tw])
            nmr_bc = scr.tile([128, T], F32, tag="bc", bufs=2, name="nmr_bc")
            nc.vector.tensor_copy(out=nmr_bc[:, :tw], in_=nmr_ps[:, :tw])
            a_src = z_sb if apply_src is None else apply_src
            for c in range(NCH):
                tmp = scr.tile([128, T], F32, tag="s4", bufs=4, name="ntmp")
                nc.vector.tensor_mul(out=tmp[:, :tw],
                                     in0=a_src[:, c, off:off + tw],
                                     in1=rstd_bc[:, :tw])
                if chunk_writer is None:
                    nc.vector.tensor_tensor(out_sb[:, c, off:off + tw], tmp[:, :tw],
                                            nmr_bc[:, :tw], ALU.add)
                else:
                    oc = scr.tile([128, T], F32, tag="s4", bufs=4, name="oc")
                    nc.vector.tensor_tensor(oc[:, :tw], tmp[:, :tw],
                                            nmr_bc[:, :tw], ALU.add)
                    chunk_writer(c, oc[:, :tw])

        # ================= attention inner block (scores/exp/AV) ===========
        # proj_spec: list of (dst, w_t, src, fold) projected per-pair right
        # before that pair's scores — keeps PE dense while ACT runs exps.
        def attn_inner(qt_sb, kt_sb, vv_sb, aot_sb, proj_spec=()):
            for p in range(NPAIR):
                for dst, w_t, src, fld in proj_spec:
                    pp = ps_tile()
                    for c in range(NCH):
                        lhsT = w_t[:, c, 128 * p:128 * (p + 1)]
                        for nh in range(2):
                            nc.tensor.matmul(
                                pp[:, 512 * nh:512 * (nh + 1)], lhsT,
                                src[:, c, 512 * nh:512 * (nh + 1)],
                                start=(c == 0), stop=(c == NCH - 1))
                    if fld is None:
                        nc.vector.tensor_copy(out=dst[:, p, :], in_=pp[:])
                    else:
                        nc.vector.tensor_scalar(
                            out=dst[:, p, :], in0=pp[:], scalar1=fld[p][:],
                            scalar2=None, op0=ALU.mult)
                av_ps = ps_tile()

                def emit_av(st, e_pair, zp):
                    rp = stat.tile([128, 2], F32, tag="rp", bufs=4, name="rp")
                    nc.vector.reciprocal(out=rp[:], in_=zp[:])
                    vv_sc = scr.tile([128, 2, 64], BF16, tag="vvsc", bufs=3,
                                     name="vv_sc")
                    nc.vector.tensor_tensor(
                        vv_sc[:],
                        vv_sb[:, st, 128 * p:128 * (p + 1)].rearrange(
                            "s (h v) -> s h v", h=2),
                        rp[:, :, None].to_broadcast([128, 2, 64]),
                        ALU.mult)
                    for h in range(2):
                        for nh in range(2):
                            nc.tensor.matmul(
                                av_ps[64 * h:64 * (h + 1),
                                      512 * nh:512 * (nh + 1)],
                                vv_sc[:, h, :],
                                e_pair[h][:, 512 * nh:512 * (nh + 1)],
                                start=(st == 0), stop=(st == NCH - 1),
                                tile_position=(0, 64 * h))

                prev = None  # one-step software pipeline: scores(st+1) issue
                for st in range(NCH):  # before AV(st)'s exp-gated wait
                    zp = stat.tile([128, 2], F32, tag="zp", bufs=4, name="zp")
                    e_pair = []
                    for h in range(2):
                        sc_ps = ps_tile()
                        k0 = 64 * h
                        lhsT = kt_sb[k0:k0 + 64, p, 128 * st:128 * (st + 1)]
                        for nh in range(2):
                            nc.tensor.matmul(
                                sc_ps[:, 512 * nh:512 * (nh + 1)], lhsT,
                                qt_sb[k0:k0 + 64, p, 512 * nh:512 * (nh + 1)],
                                start=True, stop=True, tile_position=(k0, 0))
                        e_st = epool.tile([128, T], BF16, tag="E", bufs=4,
                                          name="e_st")
                        nc.scalar.activation(
                            out=e_st[:], in_=sc_ps[:], func=AF.Exp, scale=ISQ,
                            accum_out=zp[:, h:h + 1])
                        e_pair.append(e_st)
                    if prev is not None:
                        emit_av(*prev)
                    prev = (st, e_pair, zp)
                emit_av(*prev)
                nc.vector.tensor_copy(out=aot_sb[:, p, :], in_=av_ps[:])

        # ================= projection helpers ==============================
        def proj_qk(dst, w_t, src, fold):
            """dst[:, p, :] = (W_pair^T @ src) [optionally * fold[p] rows]."""
            for p in range(NPAIR):
                pp = ps_tile()
                for c in range(NCH):
                    lhsT = w_t[:, c, 128 * p:128 * (p + 1)]
                    for nh in range(2):
                        nc.tensor.matmul(
                            pp[:, 512 * nh:512 * (nh + 1)], lhsT,
                            src[:, c, 512 * nh:512 * (nh + 1)],
                            start=(c == 0), stop=(c == NCH - 1))
                if fold is None:
                    nc.vector.tensor_copy(out=dst[:, p, :], in_=pp[:])
                else:
                    nc.vector.tensor_scalar(
                        out=dst[:, p, :], in0=pp[:], scalar1=fold[p][:],
                        scalar2=None, op0=ALU.mult)

        def proj_v(dst, w_t, src):
            for st in range(NCH):
                vp = ps_tile((128, 512))
                for c in range(NCH):
                    nc.tensor.matmul(
                        vp[:], src[:, c, 128 * st:128 * (st + 1)],
                        w_t[:, c, :], start=(c == 0), stop=(c == NCH - 1))
                nc.vector.tensor_copy(out=dst[:, st, :], in_=vp[:])

        def wo_partial(wo_sb, aot_sb, emit_e_tile):
            for e in range(NCH):
                wo_ps = ps_tile()
                for p in range(NPAIR):
                    lhsT = wo_sb[:, p, 128 * e:128 * (e + 1)]
                    for nh in range(2):
                        nc.tensor.matmul(
                            wo_ps[:, 512 * nh:512 * (nh + 1)], lhsT,
                            aot_sb[:, p, 512 * nh:512 * (nh + 1)],
                            start=(p == 0), stop=(p == NPAIR - 1))
                emit_e_tile(e, wo_ps)

        def load_w(pool, name, tag):
            t = pool.tile([128, NCH, 512], BF16, tag=tag, name=name + "_sb")
            nc.sync.dma_start(t[:], io[name].ap().rearrange("c p k -> p c k"))
            return t

        # ============================ start =================================
        with tc.tile_pool(name="actA", bufs=1) as actA:
            y_sb = actA.tile([128, NCH, T], BF16, tag="y", name="y_sb")

            with tc.tile_pool(name="gio", bufs=1) as gio:
                qt = gio.tile([128, NPAIR, T], BF16, tag="qt", name="qt")
                kt = gio.tile([128, NPAIR, T], BF16, tag="kt", name="kt")
                vv = gio.tile([128, NCH, 512], BF16, tag="vv", name="vv")
                aot = gio.tile([128, NPAIR, T], BF16, tag="aot", name="aot")

                ag_in = dram.tile([NPAIR, 128, T], BF16, tag="ag_in",
                                  name="ag_in")
                ag_out = dram.tile([2, NPAIR, 128, T], BF16, tag="ag_out",
                                   name="ag_out")
                TQ = THALF // 2
                rs_inA = dram.tile([2, NCH, 128, TQ], BF16, tag="rs_inA",
                                   name="rs_inA")
                rs_inB = dram.tile([2, NCH, 128, TQ], BF16, tag="rs_inB",
                                   name="rs_inB")
                rs_outA = dram.tile([NCH, 128, TQ], BF16, tag="rs_outA",
                                    name="rs_outA")
                rs_outB = dram.tile([NCH, 128, TQ], BF16, tag="rs_outB",
                                    name="rs_outB")

                # ---------------- Layer 1 ----------------
                with tc.tile_pool(name="w1", bufs=1) as w1:
                    # masked weight softmax for Wq1/Wk1 (no max-subtraction)
                    ewq = w1.tile([128, NCH, 512], BF16, tag="ewq", name="ewq")
                    ewk = w1.tile([128, NCH, 512], BF16, tag="ewk", name="ewk")
                    for nm, ew in (("wq1", ewq), ("wk1", ewk)):
                        raw = w1.tile([128, NCH, 512], BF16, tag="wraw",
                                      bufs=1, name="wraw")
                        nc.sync.dma_start(
                            raw[:], io[nm].ap().rearrange("c p k -> p c k"))
                        nc.scalar.activation(out=ew[:], in_=raw[:], func=AF.Exp)
                        nc.vector.tensor_tensor(
                            ew[:, 0, :].rearrange("p (q k) -> p q k", q=NPAIR),
                            ew[:, 0, :].rearrange("p (q k) -> p q k", q=NPAIR),
                            mask_sb[:, None, :].to_broadcast([128, NPAIR, 128]),
                            ALU.mult)
                    nc.sync.dma_start(
                        y_sb[:], io["yT"].ap().rearrange("c p t -> p c t"))
                    # column sums over d -> ck = 1/(Sq*Sk), transposed per pair
                    sq_ps = ps_tile((1, 512))
                    for c in range(NCH):
                        nc.tensor.matmul(sq_ps[:], ones_col[:], ewq[:, c, :],
                                         start=(c == 0), stop=(c == NCH - 1))
                    sk_ps = ps_tile((1, 512))
                    for c in range(NCH):
                        nc.tensor.matmul(sk_ps[:], ones_col[:], ewk[:, c, :],
                                         start=(c == 0), stop=(c == NCH - 1))
                    rq = stat.tile([1, 512], F32, tag="rq", name="rq")
                    rk = stat.tile([1, 512], F32, tag="rk", name="rk")
                    nc.vector.reciprocal(out=rq[:], in_=sq_ps[:])
                    nc.vector.reciprocal(out=rk[:], in_=sk_ps[:])
                    ckk = stat.tile([1, 512], F32, tag="ck", name="ckk")
                    nc.vector.tensor_mul(out=ckk[:], in0=rq[:], in1=rk[:])
                    ckT = []
                    for p in range(NPAIR):
                        ct_ps = ps_tile((128, 1))
                        nc.tensor.matmul(ct_ps[:],
                                         ckk[:, 128 * p:128 * (p + 1)],
                                         one1[:], start=True, stop=True)
                        ct = stat.tile([128, 1], F32, tag=f"ckT{p}",
                                       name=f"ckT{p}")
                        nc.vector.tensor_copy(out=ct[:], in_=ct_ps[:])
                        ckT.append(ct)

                    wv1 = load_w(w1, "wv1", "wv")
                    proj_v(vv, wv1, y_sb)

                    proj_qk(qt, ewq, y_sb, None)
                    proj_qk(kt, ewk, y_sb, ckT)
                    attn_inner(qt, kt, vv, aot)
                    _dump(nc, dbg, "d_qt1", qt[:])
                    _dump(nc, dbg, "d_kt1", kt[:])
                    _dump(nc, dbg, "d_vv1", vv[:])
                    _dump(nc, dbg, "d_aot1", aot[:])
                    nc.sync.dma_start(ag_in.rearrange("q p t -> p q t"),
                                      aot[:])
                nc.gpsimd.collective_compute(
                    "AllGather", ALU.bypass, replica_groups=REPLICA_GROUPS,
                    ins=[ag_in.opt()], outs=[ag_out.opt()])

                # ---------------- Layer 2 (overlaps the AllReduce) ---------
                with tc.tile_pool(name="actB", bufs=1) as actB:
                    x_sb = actB.tile([128, NCH, T], BF16, tag="x", name="x_sb")
                    nc.sync.dma_start(
                        x_sb[:], io["xT"].ap().rearrange("c p t -> p c t"))
                    with tc.tile_pool(name="w2", bufs=1) as w2:
                        wq2 = load_w(w2, "wq2", "wq2")
                        wk2 = load_w(w2, "wk2", "wk2")
                        wv2 = load_w(w2, "wv2", "wv2")
                        wo2 = w2.tile([128, NPAIR, D], BF16, tag="wo2",
                                      name="wo2")
                        nc.sync.dma_start(
                            wo2[:], io["wo2"].ap().rearrange("q p e -> p q e"))
                        # K/V projections only need x -> run during the AR
                        proj_qk(kt, wk2, x_sb, None)
                        proj_v(vv, wv2, x_sb)

                        # gather both cores' AOT, full Wo1 locally;
                        # z1 = Wo1(aot_full) + y(fp32), in place into y_sb
                        wo1f = w2.tile([128, 2, NPAIR, D], BF16, tag="wo1f",
                                       name="wo1f")
                        nc.sync.dma_start(
                            wo1f[:], io["wo1"].ap()
                            .rearrange("(r q) p e -> p r q e", r=2))
                        aot_full = w2.tile([128, 2, NPAIR, T], BF16,
                                           tag="aotf", name="aot_full")
                        nc.sync.dma_start(
                            aot_full[:],
                            ag_out.rearrange("r q p t -> p r q t"))
                        for e in range(NCH):
                            wo_ps = ps_tile()
                            for r in range(2):
                                for p in range(NPAIR):
                                    lhsT = wo1f[:, r, p, 128 * e:128 * (e + 1)]
                                    for nh in range(2):
                                        nc.tensor.matmul(
                                            wo_ps[:, 512 * nh:512 * (nh + 1)],
                                            lhsT,
                                            aot_full[:, r, p,
                                                     512 * nh:512 * (nh + 1)],
                                            start=(r == 0 and p == 0),
                                            stop=(r == 1 and p == NPAIR - 1))
                            yf = scr.tile([128, T], F32, tag="s4", bufs=4,
                                          name="yf")
                            nc.sync.dma_start(yf[:], io["yT32"].ap()[e])
                            nc.vector.tensor_tensor(y_sb[:, e, :], wo_ps[:],
                                                    yf[:], ALU.add)
                        out1 = y_sb  # alias: z1 normalized in place
                        norm_into(y_sb, T, out1)
                        _dump(nc, dbg, "d_out1T", out1[:])

                        proj_qk(qt, wq2, out1, None)
                        half_pre = w2.tile([128, NCH, T], BF16,
                                           tag="aotf", name="half_pre")
                        for c in range(NCH):
                            nc.vector.tensor_scalar(
                                out=half_pre[:, c, :], in0=out1[:, c, :],
                                scalar1=0.5, scalar2=None, op0=ALU.mult)
                        attn_inner(qt, kt, vv, aot)

                        def emit_rs(e, wo_ps):
                            res = scr.tile([128, T], BF16, tag="sb4", bufs=2,
                                           name="res")
                            nc.vector.tensor_tensor(res[:], wo_ps[:],
                                                    half_pre[:, e, :],
                                                    ALU.add)
                            nc.sync.dma_start(rs_inA[0, e], res[:, 0:TQ])
                            nc.sync.dma_start(rs_inA[1, e],
                                              res[:, THALF:THALF + TQ])
                            nc.sync.dma_start(rs_inB[0, e], res[:, TQ:THALF])
                            nc.sync.dma_start(rs_inB[1, e], res[:, THALF + TQ:])

                        wo_partial(wo2, aot, emit_rs)
                nc.gpsimd.collective_compute(
                    "ReduceScatter", ALU.add, replica_groups=REPLICA_GROUPS,
                    ins=[rs_inA.opt()], outs=[rs_outA.opt()])
                nc.gpsimd.collective_compute(
                    "ReduceScatter", ALU.add, replica_groups=REPLICA_GROUPS,
                    ins=[rs_inB.opt()], outs=[rs_outB.opt()])

            # ---------------- FFN on local t-half ----------------
            with tc.tile_pool(name="fh", bufs=1) as fh:
                h_sb = fh.tile([128, NFT, THALF], BF16, tag="h", name="h_sb")
                NWIN = 4
                pre_win = {}
                for ft in range(NWIN):
                    wt = fh.tile([128, NCH, 128], BF16, tag="win",
                                 bufs=NWIN, name="win_t")
                    nc.sync.dma_start(
                        wt[:], io["win"].ap()[ft].rearrange("c p f -> p c f"))
                    pre_win[ft] = wt
                pre_wout = fh.tile([128, NFT, 128], BF16, tag="wout", bufs=2,
                                   name="wout_t")
                nc.sync.dma_start(pre_wout[:],
                                  io["wout"].ap()[0].rearrange("f p m -> p f m"))

                TQ = THALF // 2
                z2 = hz.tile([128, NCH, THALF], BF16, tag="hz", bufs=1, name="z2")
                nc.sync.dma_start(z2[:, :, 0:TQ],
                                  rs_outA.rearrange("c p t -> p c t"))
                out2 = z2
                norm_into(z2, TQ, out2, off=0)
                nc.sync.dma_start(z2[:, :, TQ:THALF],
                                  rs_outB.rearrange("c p t -> p c t"))
                win_tiles = {}
                for ft in range(NFT):
                    if ft in pre_win:
                        win_t = pre_win.pop(ft)
                    else:
                        win_t = fh.tile([128, NCH, 128], BF16, tag="win",
                                        bufs=NWIN, name="win_t")
                        nc.sync.dma_start(
                            win_t[:],
                            io["win"].ap()[ft].rearrange("c p f -> p c f"))
                    win_tiles[ft] = win_t
                    hp = ps_tile((128, THALF))
                    for c in range(NCH):
                        nc.tensor.matmul(hp[:, 0:TQ], win_t[:, c, :],
                                         out2[:, c, 0:TQ],
                                         start=(c == 0), stop=(c == NCH - 1))
                    nc.scalar.activation(out=h_sb[:, ft, 0:TQ], in_=hp[:, 0:TQ],
                                         func=AF.Relu,
                                         bias=bin_sb[:, ft:ft + 1], scale=1.0)
                    if ft == 0:
                        norm_into(z2, TQ, out2, off=TQ)
                    hp2 = ps_tile((128, THALF))
                    for c in range(NCH):
                        nc.tensor.matmul(hp2[:, 0:TQ], win_t[:, c, :],
                                         out2[:, c, TQ:THALF],
                                         start=(c == 0), stop=(c == NCH - 1))
                    nc.scalar.activation(out=h_sb[:, ft, TQ:THALF],
                                         in_=hp2[:, 0:TQ], func=AF.Relu,
                                         bias=bin_sb[:, ft:ft + 1], scale=1.0)
                    del win_tiles[ft]

                z3 = out2  # in-place: out2[:, e] is last read by this add
                z3f = fh.tile([128, NCH, THALF], F32, tag="z3f", name="z3f")
                for e in range(NCH):
                    if e == 0:
                        wout_t = pre_wout
                    else:
                        wout_t = fh.tile([128, NFT, 128], BF16,
                                         tag="wout", bufs=2, name="wout_t")
                        nc.sync.dma_start(
                            wout_t[:],
                            io["wout"].ap()[e].rearrange("f p m -> p f m"))
                    fp = ps_tile((128, THALF))
                    for fc in range(NFT):
                        nc.tensor.matmul(
                            fp[:], wout_t[:, fc, :], h_sb[:, fc, :],
                            start=(fc == 0), stop=(fc == NFT - 1))
                    t1 = scr.tile([128, T], F32, tag="s4", bufs=4, name="fftmp")
                    nc.vector.tensor_scalar(out=t1[:, :THALF], in0=fp[:],
                                            scalar1=bout_sb[:, e:e + 1],
                                            scalar2=None, op0=ALU.add)
                    nc.vector.tensor_tensor(z3f[:, e, :], t1[:, :THALF],
                                            out2[:, e, :], ALU.add)
                    nc.vector.tensor_copy(out=z3[:, e, :], in_=z3f[:, e, :])
                norm_into(z3, THALF, None,
                          chunk_writer=lambda c, oc: nc.sync.dma_start(
                              out_d.ap()[c], oc), apply_src=z3f)


# ============================================================== host side ===
def _to_bf16(a):
    import ml_dtypes
    return np.asarray(a, np.float32).astype(ml_dtypes.bfloat16)


def _prep_inputs(inputs):
    """Per-core in_maps (host does transposes/tiling/dtype casts only)."""
    x = np.asarray(inputs["x"], np.float32)
    y = np.asarray(inputs["y"], np.float32)
    mask = _to_bf16(np.tile(np.tril(np.ones((128, DK), np.float32)), (1, 2)))
    win_t = _to_bf16(np.asarray(inputs["w_in"], np.float32).T
                     .reshape(NCH, 128, NFT, 128).transpose(2, 0, 1, 3))
    wout_t = _to_bf16(np.asarray(inputs["w_out"], np.float32).T
                      .reshape(NFT, 128, NCH, 128).transpose(2, 0, 1, 3))
    bin2 = np.ascontiguousarray(
        np.asarray(inputs["b_in"], np.float32).reshape(NFT, 128).T)
    bout2 = np.ascontiguousarray(
        np.asarray(inputs["b_out"], np.float32).reshape(NCH, 128).T)

    def packw(w, hs):
        return _to_bf16(np.asarray(w, np.float32)[hs].transpose(1, 0, 2)
                        .reshape(D, 512).reshape(NCH, 128, 512))

    shared = {"win": win_t, "wout": wout_t, "bin": bin2, "bout": bout2,
              "mask": mask}
    in_maps = []
    for c in range(NCORES):
        b, m = c // 2, c % 2
        hs = slice(8 * m, 8 * (m + 1))
        im = dict(shared)
        im["yT"] = _to_bf16(y[b].T.reshape(NCH, 128, T))
        im["yT32"] = np.ascontiguousarray(y[b].T.reshape(NCH, 128, T))
        im["xT"] = _to_bf16(x[b].T.reshape(NCH, 128, T))
        im["wq1"] = packw(inputs["Wq1"], hs)
        im["wk1"] = packw(inputs["Wk1"], hs)
        im["wv1"] = packw(inputs["Wv1"], hs)
        im["wq2"] = packw(inputs["Wq2"], hs)
        im["wk2"] = packw(inputs["Wk2"], hs)
        im["wv2"] = packw(inputs["Wv2"], hs)
        im["wo1"] = _to_bf16(np.asarray(inputs["Wo1"], np.float32)
                             .reshape(2 * NPAIR, 128, D))
        im["wo2"] = _to_bf16(np.asarray(inputs["Wo2"], np.float32)
                             [512 * m:512 * (m + 1)].reshape(NPAIR, 128, D))
        in_maps.append(im)
    return in_maps


def _assemble(results):
    out3 = np.empty((B, T, D), np.float32)
    for b in range(B):
        halves = [results[2 * b + m]["out3T"].reshape(D, THALF)
                  for m in range(2)]
        out3[b] = np.concatenate(halves, axis=1).T
    return out3


# ================================================================ runner ===
_CACHE = {}


def _make_runner(nc, n_cores):
    import jax
    from jax.sharding import Mesh, PartitionSpec
    from jax.experimental.shard_map import shard_map
    from concourse.bass2jax import (_bass_exec_p, install_neuronx_cc_hook,
                                    partition_id_tensor)

    install_neuronx_cc_hook()
    partition_name = nc.partition_id_tensor.name if nc.partition_id_tensor else None
    in_names, out_names, out_avals, zero_outs = [], [], [], []
    for alloc in nc.m.functions[0].allocations:
        if not isinstance(alloc, mybir.MemoryLocationSet):
            continue
        name = alloc.memorylocations[0].name
        if alloc.kind == "ExternalInput":
            if name != partition_name:
                in_names.append(name)
        elif alloc.kind == "ExternalOutput":
            shape = tuple(alloc.tensor_shape)
            dtype = mybir.dt.np(alloc.dtype)
            out_names.append(name)
            out_avals.append(jax.core.ShapedArray(shape, dtype))
            zero_outs.append(np.zeros(shape, dtype))
    n_params = len(in_names)
    n_outs = len(out_avals)
    all_in = in_names + out_names + ([partition_name] if partition_name else [])

    def _body(*args):
        operands = list(args)
        if partition_name is not None:
            operands.append(partition_id_tensor())
        return tuple(_bass_exec_p.bind(
            *operands, out_avals=tuple(out_avals), in_names=tuple(all_in),
            out_names=tuple(out_names), lowering_input_output_aliases=(),
            sim_require_finite=True, sim_require_nnan=True, nc=nc))

    devices = jax.devices()[:n_cores]
    mesh = Mesh(np.asarray(devices), ("core",))
    sharded = jax.jit(
        shard_map(_body, mesh=mesh,
                  in_specs=(PartitionSpec("core"),) * (n_params + n_outs),
                  out_specs=(PartitionSpec("core"),) * n_outs,
                  check_rep=False),
        keep_unused=True)

    def run(in_maps):
        concat_in = [
            np.concatenate([np.asarray(in_maps[c][nm]) for c in range(n_cores)],
                           axis=0)
            for nm in in_names
        ]
        concat_zero = [np.concatenate([z] * n_cores, axis=0) for z in zero_outs]
        outs = [np.asarray(o) for o in sharded(*concat_in, *concat_zero)]
        results = []
        for c in range(n_cores):
            r = {}
            for i, nm in enumerate(out_names):
                per = outs[i].shape[0] // n_cores
                r[nm] = outs[i][c * per:(c + 1) * per]
            results.append(r)
        return results

    return run


def _get_built(debug=False):
    key = "dbg" if debug else "main"
    if key not in _CACHE:
        nc = build_nc(debug=debug)
        run = _make_runner(nc, NCORES)
        _CACHE[key] = (nc, run)
    return _CACHE[key]


def kernel(**inputs):
    nc, run = _get_built()
    in_maps = _prep_inputs(inputs)
    results = run(in_maps)
    out3 = _assemble(results)
    return (np.asarray(inputs["x"], np.float32), out3)
